# revision 1
# baseline (speedup 1.0000x reference)
"""Multi-head attention (B=2, S=2048, D=1024, H=16, causal mask) on 8 TRN2 cores.

Sharding: core c handles batch b = c//4 and 4 heads g = c%4 (dims 256g..256g+256
of the projection space).  Each core computes a partial output [S, D] (its 4
heads' contribution to the out-projection); the host sums the 4 partials per
batch and adds the output bias.

Device layout (per core) keeps the sequence axis on the SBUF free dimension:
  QT, KT  [256, 2048]  (head-dim on partitions, 2 head-pairs of 128)
  V_aug   16 tiles [128, 4, 65]  (seq on partitions; per head 64 dims + ones col)
  scores  S.T tiles [128 k, 512 q] per head; causal blocks above diagonal skipped
  exp     ScalarE, scale=1/8, mask folded in as a -1e9 bias (one [128,128] tri tile)
  ctx.T   [65, 512] PSUM per (head, q-chunk); row 64 = softmax denominator l
  norm    reciprocal_approx_fast on l, partition_broadcast, DVE multiply
  out     ctxT (4 heads stacked, [256, 2048]) @ o_w slice -> [2048, 1024]
All matmuls run as float32r (f32 storage bitcast; full PE rate at N>=256).
"""

import numpy as np
from contextlib import ExitStack

import concourse.bacc as bacc
import concourse.bass as bass
import concourse.tile as tile
from concourse import mybir

P = 128
S = 2048
D = 1024
N_HEADS_TOT = 16
HEADS = 4            # per core
HD = 64
M_DIM = HEADS * HD   # 256
KC = 8               # embed-dim 128-chunks
QCW = 512            # q chunk width
NQC = S // QCW       # 4
NKT = S // P         # 16 k-tiles
F32 = mybir.dt.float32
F32R = mybir.dt.float32r
BF16 = mybir.dt.bfloat16
EXPF = mybir.ActivationFunctionType.Exp
NEG = -1.0e9

TRACE = False
LAST_RESULTS = None
_NC_CACHE = {}


def build_nc(mode: str, compile_: bool = True, probes: bool = False,
             has_bias: bool = False) -> bass.Bass:
    """mode in {causal, nomask, generic}"""
    nc = bacc.Bacc("TRN2", target_bir_lowering=False, debug=False)
    prb = {}
    if probes:
        for nm, shape in (("p_qt", [P, S]), ("p_kt", [P, S]),
                          ("p_va", [P, HEADS * (HD + 1)]), ("p_ct", [P, S])):
            prb[nm] = nc.dram_tensor(nm, shape, F32, kind="ExternalOutput").ap()
    xq = nc.dram_tensor("xqT", [D + 1, S], BF16, kind="ExternalInput").ap()
    xk = nc.dram_tensor("xkT", [D + 1, S], BF16, kind="ExternalInput").ap()
    xv = nc.dram_tensor("xvT", [D + 1, S], BF16, kind="ExternalInput").ap()
    wq = nc.dram_tensor("wqT", [D + 1, M_DIM], BF16, kind="ExternalInput").ap()
    wk = nc.dram_tensor("wkT", [D + 1, M_DIM], BF16, kind="ExternalInput").ap()
    wv = nc.dram_tensor("wvT", [D + 1, M_DIM], BF16, kind="ExternalInput").ap()
    ow = nc.dram_tensor("owT", [M_DIM, D], BF16, kind="ExternalInput").ap()
    btri = nc.dram_tensor("btri", [P, P], F32, kind="ExternalInput").ap()
    bfull = None
    if mode == "generic":
        bfull = nc.dram_tensor("biasT", [S, S], F32, kind="ExternalInput").ap()
    out = nc.dram_tensor("out", [S, D], BF16, kind="ExternalOutput").ap()

    with tile.TileContext(nc) as tc, ExitStack() as ctx:
        consts = ctx.enter_context(tc.tile_pool(name="consts", bufs=1))
        xpool = ctx.enter_context(tc.tile_pool(name="xpool", bufs=26))
        qkv = ctx.enter_context(tc.tile_pool(name="qkv", bufs=1))
        ppool = ctx.enter_context(tc.tile_pool(name="ppool", bufs=6))
        bpool = ctx.enter_context(tc.tile_pool(name="bpool", bufs=2))
        small = ctx.enter_context(tc.tile_pool(name="small", bufs=4))
        outp = ctx.enter_context(tc.tile_pool(name="outp", bufs=2))
        spool = ctx.enter_context(tc.tile_pool(name="spsum", bufs=3, space="PSUM"))
        cpool = ctx.enter_context(tc.tile_pool(name="cpsum", bufs=2, space="PSUM"))

        # ---- resident weights ----
        def load_w(ap_dram, nm):
            tiles = []
            for kc in range(KC):
                t = consts.tile([P, M_DIM], BF16, name=f"{nm}{kc}")
                nc.scalar.dma_start(out=t, in_=ap_dram[P * kc:P * (kc + 1), :])
                tiles.append(t)
            aug = None
            if has_bias:
                aug = consts.tile([1, M_DIM], BF16, name=f"{nm}_aug")
                nc.sync.dma_start(out=aug, in_=ap_dram[D:D + 1, :])
            return tiles, aug

        wq_sb, wq_aug = load_w(wq, "wq")
        wk_sb, wk_aug = load_w(wk, "wk")
        wv_sb, wv_aug = load_w(wv, "wv")
        ow_sb = []
        for pr in range(2):
            t = consts.tile([P, D], BF16, name=f"ow{pr}")
            nc.scalar.dma_start(out=t, in_=ow[P * pr:P * (pr + 1), :])
            ow_sb.append(t)
        btri_sb = consts.tile([P, P], F32, name="btri_sb")
        nc.scalar.dma_start(out=btri_sb, in_=btri)
        ones4 = consts.tile([P, HEADS], F32, name="ones4")
        nc.vector.memset(ones4, 1.0)

        QT = [qkv.tile([P, S], BF16, name=f"QT{pr}") for pr in range(2)]
        KT = [qkv.tile([P, S], BF16, name=f"KT{pr}") for pr in range(2)]
        CT = [qkv.tile([P, S], BF16, name=f"CT{pr}") for pr in range(2)]
        VA = [qkv.tile([P, HEADS, HD + 1], BF16, name=f"VA{t}") for t in range(NKT)]

        pending_outproj = []

        def emit_outproj(qc):
            for mq in range(QCW // P):
                out_sb = outp.tile([P, D], BF16, name="out_sb")
                q0 = QCW * qc + P * mq
                for ne in range(2):
                    o_ps = spool.tile([P, 2, QCW], F32, name="s_ps")[:, 0, :]
                    for pr2 in range(2):
                        nc.tensor.matmul(
                            o_ps,
                            lhsT=CT[pr2][:, q0:q0 + P],
                            rhs=ow_sb[pr2][:, QCW * ne:QCW * (ne + 1)],
                            start=(pr2 == 0), stop=(pr2 == 1))
                    nc.vector.tensor_copy(out_sb[:, QCW * ne:QCW * (ne + 1)], o_ps)
                nc.gpsimd.dma_start(out=out[q0:q0 + P, :], in_=out_sb)

        def load_pieces(xap, n):
            """8 [128, 1024] pieces of x.T covering the q columns of stages
            n and n+1, plus the bias ones-row piece."""
            ps = []
            for kc in range(KC):
                xt = xpool.tile([P, 2 * QCW], BF16, name="xt")
                nc.sync.dma_start(
                    out=xt,
                    in_=xap[P * kc:P * (kc + 1), QCW * n:QCW * (n + 2)])
                ps.append(xt)
            aug = None
            if has_bias:
                aug = small.tile([1, 2 * QCW], BF16, name="xaug", bufs=3)
                nc.sync.dma_start(
                    out=aug, in_=xap[D:D + 1, QCW * n:QCW * (n + 2)])
            return ps, aug

        xh = {}
        for n in range(NQC):
            # ---- stage n projections: q/k columns + v rows [512n, 512n+512) ----
            if n % 2 == 0:
                xh["q"] = load_pieces(xq, n)
                xh["k"] = load_pieces(xk, n)
                xh["v"] = load_pieces(xv, n)
            hs = (n % 2) * QCW  # column offset within the 2-stage piece
            for key, w_sb, w_aug, dest in (("q", wq_sb, wq_aug, QT),
                                           ("k", wk_sb, wk_aug, KT)):
                x_p, x_a = xh[key]
                for m in range(2):
                    ps = spool.tile([P, 2, QCW], F32, name="s_ps")
                    for kc in range(KC):
                        nc.tensor.matmul(
                            ps[:, 0, :],
                            lhsT=w_sb[kc][:, P * m:P * (m + 1)],
                            rhs=x_p[kc][:, hs:hs + QCW],
                            start=(kc == 0),
                            stop=(not has_bias and kc == KC - 1))
                    if has_bias:
                        nc.tensor.matmul(
                            ps[:, 0, :],
                            lhsT=w_aug[0:1, P * m:P * (m + 1)],
                            rhs=x_a[0:1, hs:hs + QCW],
                            start=False, stop=True)
                    nc.vector.tensor_copy(
                        dest[m][:, QCW * n:QCW * (n + 1)], ps[:, 0, :])
            xv_p, xv_a = xh["v"]
            for mv in range(4):
                m = 4 * n + mv
                ps = spool.tile([P, 2, QCW], F32, name="s_ps")
                for kc in range(KC):
                    nc.tensor.matmul(
                        ps[:, 0, 0:M_DIM],
                        lhsT=xv_p[kc][:, hs + P * mv:hs + P * (mv + 1)],
                        rhs=wv_sb[kc],
                        start=(kc == 0),
                        stop=(not has_bias and kc == KC - 1))
                if has_bias:
                    nc.tensor.matmul(
                        ps[:, 0, 0:M_DIM],
                        lhsT=xv_a[0:1, hs + P * mv:hs + P * (mv + 1)],
                        rhs=wv_aug,
                        start=False, stop=True)
                nc.vector.tensor_copy(
                    VA[m][:, :, 0:HD],
                    ps[:, 0, 0:M_DIM].rearrange("p (h d) -> p h d", h=HEADS))
                nc.vector.tensor_copy(
                    VA[m][:, :, HD:HD + 1],
                    ones4.rearrange("p (h o) -> p h o", o=1))
            if pending_outproj:
                emit_outproj(pending_outproj.pop(0))
            if probes and n == NQC - 1:
                nc.sync.dma_start(out=prb["p_qt"].bitcast(BF16)[:, 0:S], in_=QT[0])
                nc.sync.dma_start(out=prb["p_kt"].bitcast(BF16)[:, 0:S], in_=KT[0])
                nc.sync.dma_start(
                    out=prb["p_va"].bitcast(BF16)[:, 0:HEADS * (HD + 1)],
                    in_=VA[0].rearrange("p h d -> p (h d)"))

            # ---- stage n attention (q chunk n) ----
            qc = n
            for pr in range(2):
                nt = 4 * qc + 4 if mode == "causal" else NKT
                ctxs = [cpool.tile([HD + 1, QCW], F32, name="ctx_ps")
                        for _ in range(2)]
                queues = ([], [])

                def flush_ctx(j):
                    t0, p0, o0 = queues[j].pop(0)
                    nc.tensor.matmul(
                        ctxs[j][:, o0:],
                        lhsT=VA[t0][:, 2 * pr + j, :],
                        rhs=p0[:, j, o0:],
                        start=(t0 == 0), stop=(t0 == nt - 1),
                        skip_group_check=True)

                for t in range(nt):
                    o = max(0, P * t - QCW * qc) if mode == "causal" else 0
                    s_ps = spool.tile([P, 2, QCW], F32, name="s_ps")
                    for j in range(2):
                        nc.tensor.matmul(
                            s_ps[:, j, o:],
                            lhsT=KT[pr][HD * j:HD * (j + 1), P * t:P * (t + 1)],
                            rhs=QT[pr][HD * j:HD * (j + 1),
                                       QCW * qc + o:QCW * (qc + 1)],
                            start=True, stop=True,
                            tile_position=(HD * j, 0))
                    if mode == "causal" and t >= 4 * qc:
                        nc.vector.tensor_add(
                            s_ps[:, :, o:o + P],
                            s_ps[:, :, o:o + P],
                            btri_sb.rearrange("p (a q) -> p a q", a=1)
                            .to_broadcast([P, 2, P]))
                    elif mode == "generic":
                        bt = bpool.tile([P, QCW], F32, name="bt")
                        nc.sync.dma_start(
                            out=bt,
                            in_=bfull[P * t:P * (t + 1), QCW * qc:QCW * (qc + 1)])
                        nc.vector.tensor_add(
                            s_ps, s_ps,
                            bt.rearrange("p (a q) -> p a q", a=1)
                            .to_broadcast([P, 2, QCW]))
                    p_sb = ppool.tile([P, 2, QCW], BF16, name="p_sb")
                    nc.scalar.activation(
                        p_sb[:, :, o:], s_ps[:, :, o:], EXPF, scale=0.125)
                    for j in range(2):
                        queues[j].append((t, p_sb, o))
                    for j in range(2):
                        if len(queues[j]) > 2:
                            flush_ctx(j)
                for j in range(2):
                    while queues[j]:
                        flush_ctx(j)
                for j in range(2):
                    ctx_ps = ctxs[j]
                    l_sb = small.tile([1, QCW], F32, name="l_sb", bufs=3)
                    nc.vector.tensor_copy(l_sb, ctx_ps[HD:HD + 1, :])
                    r_sb = small.tile([1, QCW], F32, name="r_sb", bufs=3)
                    nc.vector.reciprocal_approx_fast(out=r_sb, in_=l_sb)
                    rbc = ppool.tile([HD, QCW], F32, name="rbc", bufs=2)
                    nc.gpsimd.partition_broadcast(out_ap=rbc, in_ap=r_sb)
                    nc.vector.tensor_mul(
                        CT[pr][HD * j:HD * (j + 1), QCW * qc:QCW * (qc + 1)],
                        ctx_ps[0:HD, :], rbc)

            pending_outproj.append(qc)
        emit_outproj(pending_outproj.pop(0))
        if probes:
            nc.sync.dma_start(out=prb["p_ct"].bitcast(BF16)[:, 0:S], in_=CT[0])

    if compile_:
        nc.compile()
    return nc


def _get_nc(mode, has_bias):
    key = (mode, has_bias)
    if key not in _NC_CACHE:
        _NC_CACHE[key] = build_nc(mode, has_bias=has_bias)
    return _NC_CACHE[key]


def _tri_bias():
    g = np.arange(P, dtype=np.int64)
    return np.where(g[None, :] < g[:, None], np.float32(NEG), np.float32(0.0))


def host_prep(query, key, value, attn_mask, q_w, q_b, k_w, k_b, v_w, v_b, o_w, o_b):
    """Build (mode, in_maps) for the 8 cores."""
    mask = np.asarray(attn_mask).astype(bool)
    if np.array_equal(mask, np.triu(np.ones((S, S), bool), 1)):
        mode = "causal"
    elif not mask.any():
        mode = "nomask"
    else:
        mode = "generic"

    import ml_dtypes
    bf16 = ml_dtypes.bfloat16
    ones_row = np.ones((1, S), bf16)

    def prep_x(x):
        return np.vstack([np.ascontiguousarray(x.T).astype(bf16), ones_row])

    xs = {}
    for b in range(2):
        xs[b] = (prep_x(np.asarray(query)[b]), prep_x(np.asarray(key)[b]),
                 prep_x(np.asarray(value)[b]))

    tri = _tri_bias()
    biasT = None
    if mode == "generic":
        biasT = np.ascontiguousarray(
            np.where(mask, np.float32(NEG), np.float32(0.0)).T)

    def prep_w(w, bvec, sl):
        return np.vstack([
            np.ascontiguousarray(np.asarray(w)[sl].T).astype(bf16),
            np.asarray(bvec)[sl][None, :].astype(bf16)])

    in_maps = []
    for c in range(8):
        b, g = divmod(c, 4)
        sl = slice(M_DIM * g, M_DIM * (g + 1))
        m = {
            "xqT": xs[b][0], "xkT": xs[b][1], "xvT": xs[b][2],
            "wqT": prep_w(q_w, q_b, sl),
            "wkT": prep_w(k_w, k_b, sl),
            "wvT": prep_w(v_w, v_b, sl),
            "owT": np.ascontiguousarray(np.asarray(o_w)[:, sl].T).astype(bf16),
            "btri": tri,
        }
        if mode == "generic":
            m["biasT"] = biasT
        in_maps.append(m)
    return mode, in_maps


def kernel(**inputs) -> np.ndarray:
    global LAST_RESULTS
    from concourse.bass_utils import run_bass_kernel_spmd

    mode, in_maps = host_prep(**inputs)
    has_bias = any(
        np.asarray(inputs[k]).any() for k in ("q_b", "k_b", "v_b"))
    nc = _get_nc(mode, has_bias)
    res = run_bass_kernel_spmd(nc, in_maps, core_ids=list(range(8)), trace=TRACE)
    LAST_RESULTS = res
    parts = [np.asarray(res.results[c]["out"]).astype(np.float32)
             for c in range(8)]
    o_b = np.asarray(inputs["o_b"]).astype(np.float32)
    out = np.stack([
        parts[0] + parts[1] + parts[2] + parts[3],
        parts[4] + parts[5] + parts[6] + parts[7],
    ], axis=0) + o_b[None, None, :]
    return out.astype(np.float32)



# revision 7
# speedup vs baseline: 1.0182x; 1.0182x over previous
"""Multi-head attention (B=2, S=2048, D=1024, H=16, causal mask) on 8 TRN2 cores.

Sharding: core c handles batch b = c//4 and 4 heads g = c%4 (dims 256g..256g+256
of the projection space).  Each core computes a partial output [S, D] (its 4
heads' contribution to the out-projection); the host sums the 4 partials per
batch and adds the output bias.

Device layout (per core) keeps the sequence axis on the SBUF free dimension:
  QT, KT  [256, 2048]  (head-dim on partitions, 2 head-pairs of 128)
  V_aug   16 tiles [128, 4, 65]  (seq on partitions; per head 64 dims + ones col)
  scores  S.T tiles [128 k, 512 q] per head; causal blocks above diagonal skipped
  exp     ScalarE, scale=1/8, mask folded in as a -1e9 bias (one [128,128] tri tile)
  ctx.T   [65, 512] PSUM per (head, q-chunk); row 64 = softmax denominator l
  norm    reciprocal_approx_fast on l, partition_broadcast, DVE multiply
  out     ctxT (4 heads stacked, [256, 2048]) @ o_w slice -> [2048, 1024]

The causal path software-pipelines emission: projection / out-projection matmul
"filler" quanta are woven between attention tiles so the PE never waits for the
ScalarE exp (which otherwise limits the attention phase), and the whole x input
is resident in SBUF (DMA'd up-front in need-ordered 512-column chunks).
"""

import numpy as np
from contextlib import ExitStack

import concourse.bacc as bacc
import concourse.bass as bass
import concourse.tile as tile
from concourse import mybir

P = 128
S = 2048
D = 1024
N_HEADS_TOT = 16
HEADS = 4            # per core
HD = 64
M_DIM = HEADS * HD   # 256
KC = 8               # embed-dim 128-chunks
QCW = 512            # q chunk width
NQC = S // QCW       # 4
NKT = S // P         # 16 k-tiles
F32 = mybir.dt.float32
F32R = mybir.dt.float32r
BF16 = mybir.dt.bfloat16
EXPF = mybir.ActivationFunctionType.Exp
NEG = -1.0e9

TRACE = False
LAST_RESULTS = None
_NC_CACHE = {}


class Doler:
    """Dole filler-generator quanta, in order, between primary steps."""

    def __init__(self, gens):
        self.gens = list(gens)
        self.done = 0

    def pump(self, upto=None, k=None):
        """Advance until `done` >= upto (absolute) or by k quanta."""
        if k is not None:
            upto = self.done + k
        while self.done < upto and self.gens:
            try:
                next(self.gens[0])
                self.done += 1
            except StopIteration:
                self.gens.pop(0)

    def drain(self):
        while self.gens:
            try:
                next(self.gens[0])
                self.done += 1
            except StopIteration:
                self.gens.pop(0)


def build_nc_causal(compile_: bool = True, has_bias: bool = False) -> bass.Bass:
    """Interleaved (software-pipelined) causal-mask build."""
    nc = bacc.Bacc("TRN2", target_bir_lowering=False, debug=False)
    xq = nc.dram_tensor("xqT", [D + 1, S], BF16, kind="ExternalInput").ap()
    xk = nc.dram_tensor("xkT", [D + 1, S], BF16, kind="ExternalInput").ap()
    xv = nc.dram_tensor("xvT", [D + 1, S], BF16, kind="ExternalInput").ap()
    wq = nc.dram_tensor("wqT", [D + 1, M_DIM], BF16, kind="ExternalInput").ap()
    wk = nc.dram_tensor("wkT", [D + 1, M_DIM], BF16, kind="ExternalInput").ap()
    wv = nc.dram_tensor("wvT", [D + 1, M_DIM], BF16, kind="ExternalInput").ap()
    ow = nc.dram_tensor("owT", [M_DIM, D], BF16, kind="ExternalInput").ap()
    btri = nc.dram_tensor("btri", [P, P], F32, kind="ExternalInput").ap()
    out = nc.dram_tensor("out", [S, D], BF16, kind="ExternalOutput").ap()

    with tile.TileContext(nc) as tc, ExitStack() as ctx:
        consts = ctx.enter_context(tc.tile_pool(name="consts", bufs=1))
        xpool = ctx.enter_context(tc.tile_pool(name="xpool", bufs=1))
        qkv = ctx.enter_context(tc.tile_pool(name="qkv", bufs=1))
        ppool = ctx.enter_context(tc.tile_pool(name="ppool", bufs=6))
        rpool = ctx.enter_context(tc.tile_pool(name="rpool", bufs=2))
        small = ctx.enter_context(tc.tile_pool(name="small", bufs=4))
        outp = ctx.enter_context(tc.tile_pool(name="outp", bufs=2))
        scp = ctx.enter_context(tc.tile_pool(name="scp", bufs=2, space="PSUM"))
        pjp = ctx.enter_context(tc.tile_pool(name="pjp", bufs=2, space="PSUM"))
        cxp = ctx.enter_context(tc.tile_pool(name="cxp", bufs=2, space="PSUM"))

        # ---- resident weights (scalar ring) ----
        def load_w(ap_dram, nm):
            tiles = []
            for kc in range(KC):
                t = consts.tile([P, M_DIM], BF16, name=f"{nm}{kc}")
                nc.scalar.dma_start(out=t, in_=ap_dram[P * kc:P * (kc + 1), :])
                tiles.append(t)
            aug = None
            if has_bias:
                aug = consts.tile([1, M_DIM], BF16, name=f"{nm}_aug")
                nc.scalar.dma_start(out=aug, in_=ap_dram[D:D + 1, :])
            return tiles, aug

        wq_sb, wq_aug = load_w(wq, "wq")
        wk_sb, wk_aug = load_w(wk, "wk")
        wv_sb, wv_aug = load_w(wv, "wv")
        ow_sb = []
        for pr in range(2):
            t = consts.tile([P, D], BF16, name=f"ow{pr}")
            nc.scalar.dma_start(out=t, in_=ow[P * pr:P * (pr + 1), :])
            ow_sb.append(t)
        btri_sb = consts.tile([P, P], F32, name="btri_sb")
        nc.scalar.dma_start(out=btri_sb, in_=btri)
        ones4 = consts.tile([P, HEADS], F32, name="ones4")
        nc.vector.memset(ones4, 1.0)

        # ---- resident x (sync ring for q/k, scalar ring for v),
        #      issued in need order: chunk-major ----
        xq_sb = [xpool.tile([P, S], BF16, name=f"xq{kc}") for kc in range(KC)]
        xk_sb = [xpool.tile([P, S], BF16, name=f"xk{kc}") for kc in range(KC)]
        xv_sb = [xpool.tile([P, S], BF16, name=f"xv{kc}") for kc in range(KC)]
        for n in range(NQC):
            c0, c1 = QCW * n, QCW * (n + 1)
            for tiles, xap, eng in ((xq_sb, xq, nc.sync), (xk_sb, xk, nc.sync),
                                    (xv_sb, xv, nc.scalar)):
                for kc in range(KC):
                    eng.dma_start(out=tiles[kc][:, c0:c1],
                                  in_=xap[P * kc:P * (kc + 1), c0:c1])
        xq_aug = xk_aug = xv_aug = None
        if has_bias:
            xq_aug = small.tile([1, S], BF16, name="xq_aug")
            nc.sync.dma_start(out=xq_aug, in_=xq[D:D + 1, :])
            xk_aug = small.tile([1, S], BF16, name="xk_aug")
            nc.sync.dma_start(out=xk_aug, in_=xk[D:D + 1, :])
            xv_aug = small.tile([1, S], BF16, name="xv_aug")
            nc.sync.dma_start(out=xv_aug, in_=xv[D:D + 1, :])

        QT = [qkv.tile([P, S], BF16, name=f"QT{pr}") for pr in range(2)]
        KT = [qkv.tile([P, S], BF16, name=f"KT{pr}") for pr in range(2)]
        CT = [qkv.tile([P, S], BF16, name=f"CT{pr}") for pr in range(2)]
        VA = [qkv.tile([P, HEADS, HD + 1], BF16, name=f"VA{t}") for t in range(NKT)]

        # ---- filler generators (each yield ~= 0.5-0.9us of PE work) ----
        def gen_qkproj(n, w_sb, w_aug, x_sb, x_aug, dest, m):
            c0, c1 = QCW * n, QCW * (n + 1)
            ps = pjp.tile([P, QCW], F32, name="pj_ps")
            for kc in range(KC):
                nc.tensor.matmul(
                    ps,
                    lhsT=w_sb[kc][:, P * m:P * (m + 1)],
                    rhs=x_sb[kc][:, c0:c1],
                    start=(kc == 0),
                    stop=(not has_bias and kc == KC - 1),
                    skip_group_check=True)
                if kc == 3:
                    yield
            if has_bias:
                nc.tensor.matmul(
                    ps,
                    lhsT=w_aug[0:1, P * m:P * (m + 1)],
                    rhs=x_aug[0:1, c0:c1],
                    start=False, stop=True, skip_group_check=True)
            nc.vector.tensor_copy(dest[m][:, c0:c1], ps)
            yield

        def gen_vproj(n, mv):
            hs = QCW * n + P * mv
            ps = pjp.tile([P, QCW], F32, name="pj_ps")
            for kc in range(KC):
                nc.tensor.matmul(
                    ps[:, 0:M_DIM],
                    lhsT=xv_sb[kc][:, hs:hs + P],
                    rhs=wv_sb[kc],
                    start=(kc == 0),
                    stop=(not has_bias and kc == KC - 1),
                    skip_group_check=True)
                if kc == 3:
                    yield
            if has_bias:
                nc.tensor.matmul(
                    ps[:, 0:M_DIM],
                    lhsT=xv_aug[0:1, hs:hs + P],
                    rhs=wv_aug,
                    start=False, stop=True, skip_group_check=True)
            m = 4 * n + mv
            nc.vector.tensor_copy(
                VA[m][:, :, 0:HD],
                ps[:, 0:M_DIM].rearrange("p (h d) -> p h d", h=HEADS))
            nc.vector.tensor_copy(
                VA[m][:, :, HD:HD + 1],
                ones4.rearrange("p (h o) -> p h o", o=1))
            yield

        def gen_outproj(qc, mq):
            q0 = QCW * qc + P * mq
            out_sb = outp.tile([P, D], BF16, name="out_sb")
            for ne in range(2):
                o_ps = pjp.tile([P, QCW], F32, name="pj_ps")
                for pr2 in range(2):
                    nc.tensor.matmul(
                        o_ps,
                        lhsT=CT[pr2][:, q0:q0 + P],
                        rhs=ow_sb[pr2][:, QCW * ne:QCW * (ne + 1)],
                        start=(pr2 == 0), stop=(pr2 == 1))
                nc.vector.tensor_copy(out_sb[:, QCW * ne:QCW * (ne + 1)], o_ps)
                if ne == 1:
                    nc.gpsimd.dma_start(out=out[q0:q0 + P, :], in_=out_sb)
                yield

        # ---- attention block for one (qc, pr): yields per tile / misc step ----
        def gen_attn(qc, pr, doler, needs):
            nt = 4 * qc + 4
            ctxs = [cxp.tile([HD + 1, QCW], F32, name="ctx_ps")
                    for _ in range(2)]
            queue = []

            def flush():
                t0, p0, o0 = queue.pop(0)
                for j in range(2):
                    nc.tensor.matmul(
                        ctxs[j][:, o0:],
                        lhsT=VA[t0][:, 2 * pr + j, :],
                        rhs=p0[:, j, o0:],
                        start=(t0 == 0), stop=(t0 == nt - 1),
                        skip_group_check=True)

            nfill = needs.get("total", 0)
            base = doler.done
            for t in range(nt):
                spread = base + (nfill * (t + 1) + nt - 1) // nt
                doler.pump(upto=max(spread, base + needs.get(t, 0)))
                o = max(0, P * t - QCW * qc)
                s_ps = scp.tile([P, 2, QCW], F32, name="s_ps")
                for j in range(2):
                    nc.tensor.matmul(
                        s_ps[:, j, o:],
                        lhsT=KT[pr][HD * j:HD * (j + 1), P * t:P * (t + 1)],
                        rhs=QT[pr][HD * j:HD * (j + 1),
                                   QCW * qc + o:QCW * (qc + 1)],
                        start=True, stop=True,
                        tile_position=(HD * j, 0))
                if t >= 4 * qc:
                    nc.vector.tensor_add(
                        s_ps[:, :, o:o + P],
                        s_ps[:, :, o:o + P],
                        btri_sb.rearrange("p (a q) -> p a q", a=1)
                        .to_broadcast([P, 2, P]))
                p_sb = ppool.tile([P, 2, QCW], BF16, name="p_sb")
                nc.scalar.activation(
                    p_sb[:, :, o:], s_ps[:, :, o:], EXPF, scale=0.125)
                queue.append((t, p_sb, o))
                if len(queue) > 2:
                    flush()
            while queue:
                doler.pump(k=1)
                flush()
            # normalize both head halves
            for j in range(2):
                doler.pump(k=1)
                l_sb = small.tile([1, QCW], F32, name="l_sb", bufs=3)
                nc.vector.tensor_copy(l_sb, ctxs[j][HD:HD + 1, :])
                r_sb = small.tile([1, QCW], F32, name="r_sb", bufs=3)
                nc.vector.reciprocal_approx_fast(out=r_sb, in_=l_sb)
                rbc = rpool.tile([HD, QCW], F32, name="rbc")
                nc.gpsimd.partition_broadcast(out_ap=rbc, in_ap=r_sb)
                nc.vector.tensor_mul(
                    CT[pr][HD * j:HD * (j + 1), QCW * qc:QCW * (qc + 1)],
                    ctxs[j][0:HD, :], rbc)

        # ---- stage loop ----
        for n in range(NQC):
            qc = n
            # pr0 block: fillers = qproj m0, kproj m0, vproj 0-3, qproj m1
            f0 = Doler([
                gen_qkproj(n, wq_sb, wq_aug, xq_sb, xq_aug, QT, 0),
                gen_qkproj(n, wk_sb, wk_aug, xk_sb, xk_aug, KT, 0),
                gen_vproj(n, 0), gen_vproj(n, 1),
                gen_vproj(n, 2), gen_vproj(n, 3),
                gen_qkproj(n, wq_sb, wq_aug, xq_sb, xq_aug, QT, 1),
            ])
            # minimum cumulative quanta before tile t (absolute indices)
            needs0 = {0: 2, 4 * n: 6, 4 * n + 1: 8, 4 * n + 2: 10,
                      4 * n + 3: 12, "total": 14}
            gen_attn(qc, 0, f0, needs0)
            f0.drain()

            # pr1 block: fillers = kproj m1, outproj(qc-1)
            gens1 = [gen_qkproj(n, wk_sb, wk_aug, xk_sb, xk_aug, KT, 1)]
            if n >= 1:
                gens1 += [gen_outproj(n - 1, mq) for mq in range(4)]
            f1 = Doler(gens1)
            needs1 = {4 * n: 2, "total": 2 + (8 if n >= 1 else 0)}
            gen_attn(qc, 1, f1, needs1)
            f1.drain()

        for mq in range(4):
            for _ in gen_outproj(NQC - 1, mq):
                pass

    if compile_:
        nc.compile()
    return nc


def build_nc(mode: str, compile_: bool = True, probes: bool = False,
             has_bias: bool = False) -> bass.Bass:
    """mode in {causal, nomask, generic}; causal uses the pipelined build."""
    if mode == "causal" and not probes:
        return build_nc_causal(compile_=compile_, has_bias=has_bias)
    nc = bacc.Bacc("TRN2", target_bir_lowering=False, debug=False)
    prb = {}
    if probes:
        for nm, shape in (("p_qt", [P, S]), ("p_kt", [P, S]),
                          ("p_va", [P, HEADS * (HD + 1)]), ("p_ct", [P, S])):
            prb[nm] = nc.dram_tensor(nm, shape, F32, kind="ExternalOutput").ap()
    xq = nc.dram_tensor("xqT", [D + 1, S], BF16, kind="ExternalInput").ap()
    xk = nc.dram_tensor("xkT", [D + 1, S], BF16, kind="ExternalInput").ap()
    xv = nc.dram_tensor("xvT", [D + 1, S], BF16, kind="ExternalInput").ap()
    wq = nc.dram_tensor("wqT", [D + 1, M_DIM], BF16, kind="ExternalInput").ap()
    wk = nc.dram_tensor("wkT", [D + 1, M_DIM], BF16, kind="ExternalInput").ap()
    wv = nc.dram_tensor("wvT", [D + 1, M_DIM], BF16, kind="ExternalInput").ap()
    ow = nc.dram_tensor("owT", [M_DIM, D], BF16, kind="ExternalInput").ap()
    btri = nc.dram_tensor("btri", [P, P], F32, kind="ExternalInput").ap()
    bfull = None
    if mode == "generic":
        bfull = nc.dram_tensor("biasT", [S, S], F32, kind="ExternalInput").ap()
    out = nc.dram_tensor("out", [S, D], BF16, kind="ExternalOutput").ap()

    with tile.TileContext(nc) as tc, ExitStack() as ctx:
        consts = ctx.enter_context(tc.tile_pool(name="consts", bufs=1))
        xpool = ctx.enter_context(tc.tile_pool(name="xpool", bufs=26))
        qkv = ctx.enter_context(tc.tile_pool(name="qkv", bufs=1))
        ppool = ctx.enter_context(tc.tile_pool(name="ppool", bufs=6))
        bpool = ctx.enter_context(tc.tile_pool(name="bpool", bufs=2))
        small = ctx.enter_context(tc.tile_pool(name="small", bufs=4))
        outp = ctx.enter_context(tc.tile_pool(name="outp", bufs=2))
        spool = ctx.enter_context(tc.tile_pool(name="spsum", bufs=3, space="PSUM"))
        cpool = ctx.enter_context(tc.tile_pool(name="cpsum", bufs=2, space="PSUM"))

        # ---- resident weights ----
        def load_w(ap_dram, nm):
            tiles = []
            for kc in range(KC):
                t = consts.tile([P, M_DIM], BF16, name=f"{nm}{kc}")
                nc.scalar.dma_start(out=t, in_=ap_dram[P * kc:P * (kc + 1), :])
                tiles.append(t)
            aug = None
            if has_bias:
                aug = consts.tile([1, M_DIM], BF16, name=f"{nm}_aug")
                nc.sync.dma_start(out=aug, in_=ap_dram[D:D + 1, :])
            return tiles, aug

        wq_sb, wq_aug = load_w(wq, "wq")
        wk_sb, wk_aug = load_w(wk, "wk")
        wv_sb, wv_aug = load_w(wv, "wv")
        ow_sb = []
        for pr in range(2):
            t = consts.tile([P, D], BF16, name=f"ow{pr}")
            nc.scalar.dma_start(out=t, in_=ow[P * pr:P * (pr + 1), :])
            ow_sb.append(t)
        btri_sb = consts.tile([P, P], F32, name="btri_sb")
        nc.scalar.dma_start(out=btri_sb, in_=btri)
        ones4 = consts.tile([P, HEADS], F32, name="ones4")
        nc.vector.memset(ones4, 1.0)

        QT = [qkv.tile([P, S], BF16, name=f"QT{pr}") for pr in range(2)]
        KT = [qkv.tile([P, S], BF16, name=f"KT{pr}") for pr in range(2)]
        CT = [qkv.tile([P, S], BF16, name=f"CT{pr}") for pr in range(2)]
        VA = [qkv.tile([P, HEADS, HD + 1], BF16, name=f"VA{t}") for t in range(NKT)]

        pending_outproj = []

        def emit_outproj(qc):
            for mq in range(QCW // P):
                out_sb = outp.tile([P, D], BF16, name="out_sb")
                q0 = QCW * qc + P * mq
                for ne in range(2):
                    o_ps = spool.tile([P, 2, QCW], F32, name="s_ps")[:, 0, :]
                    for pr2 in range(2):
                        nc.tensor.matmul(
                            o_ps,
                            lhsT=CT[pr2][:, q0:q0 + P],
                            rhs=ow_sb[pr2][:, QCW * ne:QCW * (ne + 1)],
                            start=(pr2 == 0), stop=(pr2 == 1))
                    nc.vector.tensor_copy(out_sb[:, QCW * ne:QCW * (ne + 1)], o_ps)
                nc.gpsimd.dma_start(out=out[q0:q0 + P, :], in_=out_sb)

        def load_pieces(xap, n):
            """8 [128, 1024] pieces of x.T covering the q columns of stages
            n and n+1, plus the bias ones-row piece."""
            ps = []
            for kc in range(KC):
                xt = xpool.tile([P, 2 * QCW], BF16, name="xt")
                nc.sync.dma_start(
                    out=xt,
                    in_=xap[P * kc:P * (kc + 1), QCW * n:QCW * (n + 2)])
                ps.append(xt)
            aug = None
            if has_bias:
                aug = small.tile([1, 2 * QCW], BF16, name="xaug", bufs=3)
                nc.sync.dma_start(
                    out=aug, in_=xap[D:D + 1, QCW * n:QCW * (n + 2)])
            return ps, aug

        xh = {}
        for n in range(NQC):
            # ---- stage n projections: q/k columns + v rows [512n, 512n+512) ----
            if n % 2 == 0:
                xh["q"] = load_pieces(xq, n)
                xh["k"] = load_pieces(xk, n)
                xh["v"] = load_pieces(xv, n)
            hs = (n % 2) * QCW  # column offset within the 2-stage piece
            for key, w_sb, w_aug, dest in (("q", wq_sb, wq_aug, QT),
                                           ("k", wk_sb, wk_aug, KT)):
                x_p, x_a = xh[key]
                for m in range(2):
                    ps = spool.tile([P, 2, QCW], F32, name="s_ps")
                    for kc in range(KC):
                        nc.tensor.matmul(
                            ps[:, 0, :],
                            lhsT=w_sb[kc][:, P * m:P * (m + 1)],
                            rhs=x_p[kc][:, hs:hs + QCW],
                            start=(kc == 0),
                            stop=(not has_bias and kc == KC - 1))
                    if has_bias:
                        nc.tensor.matmul(
                            ps[:, 0, :],
                            lhsT=w_aug[0:1, P * m:P * (m + 1)],
                            rhs=x_a[0:1, hs:hs + QCW],
                            start=False, stop=True)
                    nc.vector.tensor_copy(
                        dest[m][:, QCW * n:QCW * (n + 1)], ps[:, 0, :])
            xv_p, xv_a = xh["v"]
            for mv in range(4):
                m = 4 * n + mv
                ps = spool.tile([P, 2, QCW], F32, name="s_ps")
                for kc in range(KC):
                    nc.tensor.matmul(
                        ps[:, 0, 0:M_DIM],
                        lhsT=xv_p[kc][:, hs + P * mv:hs + P * (mv + 1)],
                        rhs=wv_sb[kc],
                        start=(kc == 0),
                        stop=(not has_bias and kc == KC - 1))
                if has_bias:
                    nc.tensor.matmul(
                        ps[:, 0, 0:M_DIM],
                        lhsT=xv_a[0:1, hs + P * mv:hs + P * (mv + 1)],
                        rhs=wv_aug,
                        start=False, stop=True)
                nc.vector.tensor_copy(
                    VA[m][:, :, 0:HD],
                    ps[:, 0, 0:M_DIM].rearrange("p (h d) -> p h d", h=HEADS))
                nc.vector.tensor_copy(
                    VA[m][:, :, HD:HD + 1],
                    ones4.rearrange("p (h o) -> p h o", o=1))
            if pending_outproj:
                emit_outproj(pending_outproj.pop(0))
            if probes and n == NQC - 1:
                nc.sync.dma_start(out=prb["p_qt"].bitcast(BF16)[:, 0:S], in_=QT[0])
                nc.sync.dma_start(out=prb["p_kt"].bitcast(BF16)[:, 0:S], in_=KT[0])
                nc.sync.dma_start(
                    out=prb["p_va"].bitcast(BF16)[:, 0:HEADS * (HD + 1)],
                    in_=VA[0].rearrange("p h d -> p (h d)"))

            # ---- stage n attention (q chunk n) ----
            qc = n
            for pr in range(2):
                nt = 4 * qc + 4 if mode == "causal" else NKT
                ctxs = [cpool.tile([HD + 1, QCW], F32, name="ctx_ps")
                        for _ in range(2)]
                queues = ([], [])

                def flush_ctx(j):
                    t0, p0, o0 = queues[j].pop(0)
                    nc.tensor.matmul(
                        ctxs[j][:, o0:],
                        lhsT=VA[t0][:, 2 * pr + j, :],
                        rhs=p0[:, j, o0:],
                        start=(t0 == 0), stop=(t0 == nt - 1),
                        skip_group_check=True)

                for t in range(nt):
                    o = max(0, P * t - QCW * qc) if mode == "causal" else 0
                    s_ps = spool.tile([P, 2, QCW], F32, name="s_ps")
                    for j in range(2):
                        nc.tensor.matmul(
                            s_ps[:, j, o:],
                            lhsT=KT[pr][HD * j:HD * (j + 1), P * t:P * (t + 1)],
                            rhs=QT[pr][HD * j:HD * (j + 1),
                                       QCW * qc + o:QCW * (qc + 1)],
                            start=True, stop=True,
                            tile_position=(HD * j, 0))
                    if mode == "causal" and t >= 4 * qc:
                        nc.vector.tensor_add(
                            s_ps[:, :, o:o + P],
                            s_ps[:, :, o:o + P],
                            btri_sb.rearrange("p (a q) -> p a q", a=1)
                            .to_broadcast([P, 2, P]))
                    elif mode == "generic":
                        bt = bpool.tile([P, QCW], F32, name="bt")
                        nc.sync.dma_start(
                            out=bt,
                            in_=bfull[P * t:P * (t + 1), QCW * qc:QCW * (qc + 1)])
                        nc.vector.tensor_add(
                            s_ps, s_ps,
                            bt.rearrange("p (a q) -> p a q", a=1)
                            .to_broadcast([P, 2, QCW]))
                    p_sb = ppool.tile([P, 2, QCW], BF16, name="p_sb")
                    nc.scalar.activation(
                        p_sb[:, :, o:], s_ps[:, :, o:], EXPF, scale=0.125)
                    for j in range(2):
                        queues[j].append((t, p_sb, o))
                    for j in range(2):
                        if len(queues[j]) > 2:
                            flush_ctx(j)
                for j in range(2):
                    while queues[j]:
                        flush_ctx(j)
                for j in range(2):
                    ctx_ps = ctxs[j]
                    l_sb = small.tile([1, QCW], F32, name="l_sb", bufs=3)
                    nc.vector.tensor_copy(l_sb, ctx_ps[HD:HD + 1, :])
                    r_sb = small.tile([1, QCW], F32, name="r_sb", bufs=3)
                    nc.vector.reciprocal_approx_fast(out=r_sb, in_=l_sb)
                    rbc = ppool.tile([HD, QCW], F32, name="rbc", bufs=2)
                    nc.gpsimd.partition_broadcast(out_ap=rbc, in_ap=r_sb)
                    nc.vector.tensor_mul(
                        CT[pr][HD * j:HD * (j + 1), QCW * qc:QCW * (qc + 1)],
                        ctx_ps[0:HD, :], rbc)

            pending_outproj.append(qc)
        emit_outproj(pending_outproj.pop(0))
        if probes:
            nc.sync.dma_start(out=prb["p_ct"].bitcast(BF16)[:, 0:S], in_=CT[0])

    if compile_:
        nc.compile()
    return nc


def _get_nc(mode, has_bias):
    key = (mode, has_bias)
    if key not in _NC_CACHE:
        _NC_CACHE[key] = build_nc(mode, has_bias=has_bias)
    return _NC_CACHE[key]


def _tri_bias():
    g = np.arange(P, dtype=np.int64)
    return np.where(g[None, :] < g[:, None], np.float32(NEG), np.float32(0.0))


def host_prep(query, key, value, attn_mask, q_w, q_b, k_w, k_b, v_w, v_b, o_w, o_b):
    """Build (mode, in_maps) for the 8 cores."""
    mask = np.asarray(attn_mask).astype(bool)
    if np.array_equal(mask, np.triu(np.ones((S, S), bool), 1)):
        mode = "causal"
    elif not mask.any():
        mode = "nomask"
    else:
        mode = "generic"

    import ml_dtypes
    bf16 = ml_dtypes.bfloat16
    ones_row = np.ones((1, S), bf16)

    def prep_x(x):
        return np.vstack([np.ascontiguousarray(x.T).astype(bf16), ones_row])

    xs = {}
    for b in range(2):
        xs[b] = (prep_x(np.asarray(query)[b]), prep_x(np.asarray(key)[b]),
                 prep_x(np.asarray(value)[b]))

    tri = _tri_bias()
    biasT = None
    if mode == "generic":
        biasT = np.ascontiguousarray(
            np.where(mask, np.float32(NEG), np.float32(0.0)).T)

    def prep_w(w, bvec, sl):
        return np.vstack([
            np.ascontiguousarray(np.asarray(w)[sl].T).astype(bf16),
            np.asarray(bvec)[sl][None, :].astype(bf16)])

    in_maps = []
    for c in range(8):
        b, g = divmod(c, 4)
        sl = slice(M_DIM * g, M_DIM * (g + 1))
        m = {
            "xqT": xs[b][0], "xkT": xs[b][1], "xvT": xs[b][2],
            "wqT": prep_w(q_w, q_b, sl),
            "wkT": prep_w(k_w, k_b, sl),
            "wvT": prep_w(v_w, v_b, sl),
            "owT": np.ascontiguousarray(np.asarray(o_w)[:, sl].T).astype(bf16),
            "btri": tri,
        }
        if mode == "generic":
            m["biasT"] = biasT
        in_maps.append(m)
    return mode, in_maps


def kernel(**inputs) -> np.ndarray:
    global LAST_RESULTS
    from concourse.bass_utils import run_bass_kernel_spmd

    mode, in_maps = host_prep(**inputs)
    has_bias = any(
        np.asarray(inputs[k]).any() for k in ("q_b", "k_b", "v_b"))
    nc = _get_nc(mode, has_bias)
    res = run_bass_kernel_spmd(nc, in_maps, core_ids=list(range(8)), trace=TRACE)
    LAST_RESULTS = res
    parts = [np.asarray(res.results[c]["out"]).astype(np.float32)
             for c in range(8)]
    o_b = np.asarray(inputs["o_b"]).astype(np.float32)
    out = np.stack([
        parts[0] + parts[1] + parts[2] + parts[3],
        parts[4] + parts[5] + parts[6] + parts[7],
    ], axis=0) + o_b[None, None, :]
    return out.astype(np.float32)


# revision 9
# speedup vs baseline: 1.1535x; 1.1330x over previous
"""Multi-head attention (B=2, S=2048, D=1024, H=16, causal mask) on 8 TRN2 cores.

Sharding: core c handles batch b = c//4 and 4 heads g = c%4 (dims 256g..256g+256
of the projection space).  Each core computes a partial output [S, D] (its 4
heads' contribution to the out-projection); the host sums the 4 partials per
batch and adds the output bias.

Device layout (per core) keeps the sequence axis on the SBUF free dimension:
  QT, KT  [256, 2048]  (head-dim on partitions, 2 head-pairs of 128)
  V_aug   16 tiles [128, 4, 65]  (seq on partitions; per head 64 dims + ones col)
  scores  S.T tiles [128 k, 512 q] per head; causal blocks above diagonal skipped
  exp     ScalarE, scale=1/8, mask folded in as a -1e9 bias (one [128,128] tri tile)
  ctx.T   [65, 512] PSUM per (head, q-chunk); row 64 = softmax denominator l
  norm    reciprocal_approx_fast on l, partition_broadcast, DVE multiply
  out     ctxT (4 heads stacked, [256, 2048]) @ o_w slice -> [2048, 1024]

The causal path software-pipelines emission: projection / out-projection matmul
"filler" quanta are woven between attention tiles so the PE never waits for the
ScalarE exp (which otherwise limits the attention phase), and the whole x input
is resident in SBUF (DMA'd up-front in need-ordered 512-column chunks).
"""

import numpy as np
from contextlib import ExitStack

import concourse.bacc as bacc
import concourse.bass as bass
import concourse.tile as tile
from concourse import mybir

P = 128
S = 2048
D = 1024
N_HEADS_TOT = 16
HEADS = 4            # per core
HD = 64
M_DIM = HEADS * HD   # 256
KC = 8               # embed-dim 128-chunks
QCW = 512            # q chunk width
NQC = S // QCW       # 4
NKT = S // P         # 16 k-tiles
F32 = mybir.dt.float32
F32R = mybir.dt.float32r
BF16 = mybir.dt.bfloat16
EXPF = mybir.ActivationFunctionType.Exp
NEG = -1.0e9

TRACE = False
LAST_RESULTS = None
_NC_CACHE = {}


class Doler:
    """Dole filler-generator quanta, in order, between primary steps."""

    def __init__(self, gens):
        self.gens = list(gens)
        self.done = 0

    def pump(self, upto=None, k=None):
        """Advance until `done` >= upto (absolute) or by k quanta."""
        if k is not None:
            upto = self.done + k
        while self.done < upto and self.gens:
            try:
                next(self.gens[0])
                self.done += 1
            except StopIteration:
                self.gens.pop(0)

    def drain(self):
        while self.gens:
            try:
                next(self.gens[0])
                self.done += 1
            except StopIteration:
                self.gens.pop(0)


def build_nc_causal(compile_: bool = True, has_bias: bool = False) -> bass.Bass:
    """Interleaved (software-pipelined) causal-mask build."""
    nc = bacc.Bacc("TRN2", target_bir_lowering=False, debug=False)
    xq = nc.dram_tensor("xqT", [D + 1, S], BF16, kind="ExternalInput").ap()
    xk = nc.dram_tensor("xkT", [D + 1, S], BF16, kind="ExternalInput").ap()
    xv = nc.dram_tensor("xvT", [D + 1, S], BF16, kind="ExternalInput").ap()
    wq = nc.dram_tensor("wqT", [D + 1, M_DIM], BF16, kind="ExternalInput").ap()
    wk = nc.dram_tensor("wkT", [D + 1, M_DIM], BF16, kind="ExternalInput").ap()
    wv = nc.dram_tensor("wvT", [D + 1, M_DIM], BF16, kind="ExternalInput").ap()
    ow = nc.dram_tensor("owT", [M_DIM, D], BF16, kind="ExternalInput").ap()
    btri = nc.dram_tensor("btri", [P, P], F32, kind="ExternalInput").ap()
    out = nc.dram_tensor("out", [S, D], BF16, kind="ExternalOutput").ap()

    with tile.TileContext(nc) as tc, ExitStack() as ctx:
        consts = ctx.enter_context(tc.tile_pool(name="consts", bufs=1))
        xpool = ctx.enter_context(tc.tile_pool(name="xpool", bufs=1))
        qkv = ctx.enter_context(tc.tile_pool(name="qkv", bufs=1))
        ppool = ctx.enter_context(tc.tile_pool(name="ppool", bufs=6))
        rpool = ctx.enter_context(tc.tile_pool(name="rpool", bufs=2))
        small = ctx.enter_context(tc.tile_pool(name="small", bufs=4))
        outp = ctx.enter_context(tc.tile_pool(name="outp", bufs=2))
        scp = ctx.enter_context(tc.tile_pool(name="scp", bufs=2, space="PSUM"))
        pjp = ctx.enter_context(tc.tile_pool(name="pjp", bufs=2, space="PSUM"))
        cxp = ctx.enter_context(tc.tile_pool(name="cxp", bufs=2, space="PSUM"))

        # ---- resident weights + xv on the gpsimd (SWDGE) queue, in need
        #      order, keeping the two HWDGE rings dedicated to xq / xk.
        #      Combined DMAs spread their descriptors over all 16 SDMA
        #      engines, so few big transfers beat many small ones. ----
        wq_all = consts.tile([P, KC, M_DIM], BF16, name="wq_all")
        nc.gpsimd.dma_start(
            out=wq_all, in_=wq[0:D, :].rearrange("(kc p) m -> p kc m", p=P))
        wk_all = consts.tile([P, KC, M_DIM], BF16, name="wk_all")
        nc.gpsimd.dma_start(
            out=wk_all, in_=wk[0:D, :].rearrange("(kc p) m -> p kc m", p=P))
        btri_sb = consts.tile([P, P], F32, name="btri_sb")
        nc.gpsimd.dma_start(out=btri_sb, in_=btri)
        wv_all = consts.tile([P, KC, M_DIM], BF16, name="wv_all")
        nc.gpsimd.dma_start(
            out=wv_all, in_=wv[0:D, :].rearrange("(kc p) m -> p kc m", p=P))
        xq_all = xpool.tile([P, KC, S], BF16, name="xq_all")
        xk_all = xpool.tile([P, KC, S], BF16, name="xk_all")
        xv_all = xpool.tile([P, KC, S], BF16, name="xv_all")
        nc.gpsimd.dma_start(
            out=xv_all[:, :, 0:S // 2],
            in_=xv[0:D, 0:S // 2].rearrange("(kc p) s -> p kc s", p=P))
        ow_all = consts.tile([P, 2, D], BF16, name="ow_all")
        nc.gpsimd.dma_start(
            out=ow_all, in_=ow[0:M_DIM, :].rearrange("(pr p) e -> p pr e", p=P))
        nc.gpsimd.dma_start(
            out=xv_all[:, :, S // 2:S],
            in_=xv[0:D, S // 2:S].rearrange("(kc p) s -> p kc s", p=P))
        wq_sb = [wq_all[:, kc, :] for kc in range(KC)]
        wk_sb = [wk_all[:, kc, :] for kc in range(KC)]
        wv_sb = [wv_all[:, kc, :] for kc in range(KC)]
        ow_sb = [ow_all[:, pr, :] for pr in range(2)]
        wq_aug = wk_aug = wv_aug = None
        if has_bias:
            wq_aug = consts.tile([1, M_DIM], BF16, name="wq_aug")
            nc.gpsimd.dma_start(out=wq_aug, in_=wq[D:D + 1, :])
            wk_aug = consts.tile([1, M_DIM], BF16, name="wk_aug")
            nc.gpsimd.dma_start(out=wk_aug, in_=wk[D:D + 1, :])
            wv_aug = consts.tile([1, M_DIM], BF16, name="wv_aug")
            nc.gpsimd.dma_start(out=wv_aug, in_=wv[D:D + 1, :])
        ones4 = consts.tile([P, HEADS], F32, name="ones4")
        nc.vector.memset(ones4, 1.0)

        # ---- resident xq / xk: one combined [128, 8kc, 1024] DMA per
        #      half-column range per key on the HWDGE rings ----
        for c0, c1 in ((0, S // 2), (S // 2, S)):
            for xall, xap, eng in ((xq_all, xq, nc.sync), (xk_all, xk, nc.scalar)):
                eng.dma_start(
                    out=xall[:, :, c0:c1],
                    in_=xap[0:D, c0:c1].rearrange("(kc p) s -> p kc s", p=P))
        xq_sb = [xq_all[:, kc, :] for kc in range(KC)]
        xk_sb = [xk_all[:, kc, :] for kc in range(KC)]
        xv_sb = [xv_all[:, kc, :] for kc in range(KC)]
        xq_aug = xk_aug = xv_aug = None
        if has_bias:
            xq_aug = small.tile([1, S], BF16, name="xq_aug")
            nc.sync.dma_start(out=xq_aug, in_=xq[D:D + 1, :])
            xk_aug = small.tile([1, S], BF16, name="xk_aug")
            nc.sync.dma_start(out=xk_aug, in_=xk[D:D + 1, :])
            xv_aug = small.tile([1, S], BF16, name="xv_aug")
            nc.sync.dma_start(out=xv_aug, in_=xv[D:D + 1, :])

        QT = [qkv.tile([P, S], BF16, name=f"QT{pr}") for pr in range(2)]
        KT = [qkv.tile([P, S], BF16, name=f"KT{pr}") for pr in range(2)]
        CT = [qkv.tile([P, S], BF16, name=f"CT{pr}") for pr in range(2)]
        VA = [qkv.tile([P, HEADS, HD + 1], BF16, name=f"VA{t}") for t in range(NKT)]

        # ---- filler generators (each yield ~= 0.5-0.9us of PE work) ----
        def gen_qkproj(n, w_sb, w_aug, x_sb, x_aug, dest, m):
            c0, c1 = QCW * n, QCW * (n + 1)
            ps = pjp.tile([P, QCW], F32, name="pj_ps")
            for kc in range(KC):
                nc.tensor.matmul(
                    ps,
                    lhsT=w_sb[kc][:, P * m:P * (m + 1)],
                    rhs=x_sb[kc][:, c0:c1],
                    start=(kc == 0),
                    stop=(not has_bias and kc == KC - 1),
                    skip_group_check=True)
                if kc == 3:
                    yield
            if has_bias:
                nc.tensor.matmul(
                    ps,
                    lhsT=w_aug[0:1, P * m:P * (m + 1)],
                    rhs=x_aug[0:1, c0:c1],
                    start=False, stop=True, skip_group_check=True)
            nc.vector.tensor_copy(dest[m][:, c0:c1], ps)
            yield

        def gen_vproj(n, mv):
            hs = QCW * n + P * mv
            ps = pjp.tile([P, QCW], F32, name="pj_ps")
            for kc in range(KC):
                nc.tensor.matmul(
                    ps[:, 0:M_DIM],
                    lhsT=xv_sb[kc][:, hs:hs + P],
                    rhs=wv_sb[kc],
                    start=(kc == 0),
                    stop=(not has_bias and kc == KC - 1),
                    skip_group_check=True)
                if kc == 3:
                    yield
            if has_bias:
                nc.tensor.matmul(
                    ps[:, 0:M_DIM],
                    lhsT=xv_aug[0:1, hs:hs + P],
                    rhs=wv_aug,
                    start=False, stop=True, skip_group_check=True)
            m = 4 * n + mv
            nc.vector.tensor_copy(
                VA[m][:, :, 0:HD],
                ps[:, 0:M_DIM].rearrange("p (h d) -> p h d", h=HEADS))
            nc.vector.tensor_copy(
                VA[m][:, :, HD:HD + 1],
                ones4.rearrange("p (h o) -> p h o", o=1))
            yield

        def gen_outproj(qc, mq):
            q0 = QCW * qc + P * mq
            out_sb = outp.tile([P, D], BF16, name="out_sb")
            for ne in range(2):
                o_ps = pjp.tile([P, QCW], F32, name="pj_ps")
                for pr2 in range(2):
                    nc.tensor.matmul(
                        o_ps,
                        lhsT=CT[pr2][:, q0:q0 + P],
                        rhs=ow_sb[pr2][:, QCW * ne:QCW * (ne + 1)],
                        start=(pr2 == 0), stop=(pr2 == 1))
                nc.vector.tensor_copy(out_sb[:, QCW * ne:QCW * (ne + 1)], o_ps)
                if ne == 1:
                    nc.gpsimd.dma_start(out=out[q0:q0 + P, :], in_=out_sb)
                yield

        # ---- attention block for one (qc, pr): yields per tile / misc step ----
        def gen_attn(qc, pr, doler, needs):
            nt = 4 * qc + 4
            ctxs = [cxp.tile([HD + 1, QCW], F32, name="ctx_ps")
                    for _ in range(2)]
            queue = []

            def flush():
                t0, p0, o0 = queue.pop(0)
                for j in range(2):
                    nc.tensor.matmul(
                        ctxs[j][:, o0:],
                        lhsT=VA[t0][:, 2 * pr + j, :],
                        rhs=p0[:, j, o0:],
                        start=(t0 == 0), stop=(t0 == nt - 1),
                        skip_group_check=True)

            nfill = needs.get("total", 0)
            base = doler.done
            for t in range(nt):
                spread = base + (nfill * (t + 1) + nt - 1) // nt
                doler.pump(upto=max(spread, base + needs.get(t, 0)))
                o = max(0, P * t - QCW * qc)
                s_ps = scp.tile([P, 2, QCW], F32, name="s_ps")
                for j in range(2):
                    nc.tensor.matmul(
                        s_ps[:, j, o:],
                        lhsT=KT[pr][HD * j:HD * (j + 1), P * t:P * (t + 1)],
                        rhs=QT[pr][HD * j:HD * (j + 1),
                                   QCW * qc + o:QCW * (qc + 1)],
                        start=True, stop=True,
                        tile_position=(HD * j, 0))
                if t >= 4 * qc:
                    nc.vector.tensor_add(
                        s_ps[:, :, o:o + P],
                        s_ps[:, :, o:o + P],
                        btri_sb.rearrange("p (a q) -> p a q", a=1)
                        .to_broadcast([P, 2, P]))
                p_sb = ppool.tile([P, 2, QCW], BF16, name="p_sb")
                nc.scalar.activation(
                    p_sb[:, :, o:], s_ps[:, :, o:], EXPF, scale=0.125)
                queue.append((t, p_sb, o))
                if len(queue) > 2:
                    flush()
            while queue:
                doler.pump(k=1)
                flush()
            # normalize both head halves
            for j in range(2):
                doler.pump(k=1)
                l_sb = small.tile([1, QCW], F32, name="l_sb", bufs=3)
                nc.vector.tensor_copy(l_sb, ctxs[j][HD:HD + 1, :])
                r_sb = small.tile([1, QCW], F32, name="r_sb", bufs=3)
                nc.vector.reciprocal_approx_fast(out=r_sb, in_=l_sb)
                rbc = rpool.tile([HD, QCW], F32, name="rbc")
                nc.gpsimd.partition_broadcast(out_ap=rbc, in_ap=r_sb)
                nc.vector.tensor_mul(
                    CT[pr][HD * j:HD * (j + 1), QCW * qc:QCW * (qc + 1)],
                    ctxs[j][0:HD, :], rbc)

        # ---- stage loop ----
        for n in range(NQC):
            qc = n
            # pr0 block: fillers = qproj m0, kproj m0, vproj 0-3, qproj m1
            f0 = Doler([
                gen_qkproj(n, wq_sb, wq_aug, xq_sb, xq_aug, QT, 0),
                gen_qkproj(n, wk_sb, wk_aug, xk_sb, xk_aug, KT, 0),
                gen_vproj(n, 0), gen_vproj(n, 1),
                gen_vproj(n, 2), gen_vproj(n, 3),
                gen_qkproj(n, wq_sb, wq_aug, xq_sb, xq_aug, QT, 1),
            ])
            # minimum cumulative quanta before tile t (absolute indices)
            needs0 = {0: 2, 4 * n: 6, 4 * n + 1: 8, 4 * n + 2: 10,
                      4 * n + 3: 12, "total": 14}
            gen_attn(qc, 0, f0, needs0)
            f0.drain()

            # pr1 block: fillers = kproj m1, outproj(qc-1)
            gens1 = [gen_qkproj(n, wk_sb, wk_aug, xk_sb, xk_aug, KT, 1)]
            if n >= 1:
                gens1 += [gen_outproj(n - 1, mq) for mq in range(4)]
            f1 = Doler(gens1)
            needs1 = {4 * n: 2, "total": 2 + (8 if n >= 1 else 0)}
            gen_attn(qc, 1, f1, needs1)
            f1.drain()

        for mq in range(4):
            for _ in gen_outproj(NQC - 1, mq):
                pass

    if compile_:
        nc.compile()
    return nc


def build_nc(mode: str, compile_: bool = True, probes: bool = False,
             has_bias: bool = False) -> bass.Bass:
    """mode in {causal, nomask, generic}; causal uses the pipelined build."""
    if mode == "causal" and not probes:
        return build_nc_causal(compile_=compile_, has_bias=has_bias)
    nc = bacc.Bacc("TRN2", target_bir_lowering=False, debug=False)
    prb = {}
    if probes:
        for nm, shape in (("p_qt", [P, S]), ("p_kt", [P, S]),
                          ("p_va", [P, HEADS * (HD + 1)]), ("p_ct", [P, S])):
            prb[nm] = nc.dram_tensor(nm, shape, F32, kind="ExternalOutput").ap()
    xq = nc.dram_tensor("xqT", [D + 1, S], BF16, kind="ExternalInput").ap()
    xk = nc.dram_tensor("xkT", [D + 1, S], BF16, kind="ExternalInput").ap()
    xv = nc.dram_tensor("xvT", [D + 1, S], BF16, kind="ExternalInput").ap()
    wq = nc.dram_tensor("wqT", [D + 1, M_DIM], BF16, kind="ExternalInput").ap()
    wk = nc.dram_tensor("wkT", [D + 1, M_DIM], BF16, kind="ExternalInput").ap()
    wv = nc.dram_tensor("wvT", [D + 1, M_DIM], BF16, kind="ExternalInput").ap()
    ow = nc.dram_tensor("owT", [M_DIM, D], BF16, kind="ExternalInput").ap()
    btri = nc.dram_tensor("btri", [P, P], F32, kind="ExternalInput").ap()
    bfull = None
    if mode == "generic":
        bfull = nc.dram_tensor("biasT", [S, S], F32, kind="ExternalInput").ap()
    out = nc.dram_tensor("out", [S, D], BF16, kind="ExternalOutput").ap()

    with tile.TileContext(nc) as tc, ExitStack() as ctx:
        consts = ctx.enter_context(tc.tile_pool(name="consts", bufs=1))
        xpool = ctx.enter_context(tc.tile_pool(name="xpool", bufs=26))
        qkv = ctx.enter_context(tc.tile_pool(name="qkv", bufs=1))
        ppool = ctx.enter_context(tc.tile_pool(name="ppool", bufs=6))
        bpool = ctx.enter_context(tc.tile_pool(name="bpool", bufs=2))
        small = ctx.enter_context(tc.tile_pool(name="small", bufs=4))
        outp = ctx.enter_context(tc.tile_pool(name="outp", bufs=2))
        spool = ctx.enter_context(tc.tile_pool(name="spsum", bufs=3, space="PSUM"))
        cpool = ctx.enter_context(tc.tile_pool(name="cpsum", bufs=2, space="PSUM"))

        # ---- resident weights ----
        def load_w(ap_dram, nm):
            tiles = []
            for kc in range(KC):
                t = consts.tile([P, M_DIM], BF16, name=f"{nm}{kc}")
                nc.scalar.dma_start(out=t, in_=ap_dram[P * kc:P * (kc + 1), :])
                tiles.append(t)
            aug = None
            if has_bias:
                aug = consts.tile([1, M_DIM], BF16, name=f"{nm}_aug")
                nc.sync.dma_start(out=aug, in_=ap_dram[D:D + 1, :])
            return tiles, aug

        wq_sb, wq_aug = load_w(wq, "wq")
        wk_sb, wk_aug = load_w(wk, "wk")
        wv_sb, wv_aug = load_w(wv, "wv")
        ow_sb = []
        for pr in range(2):
            t = consts.tile([P, D], BF16, name=f"ow{pr}")
            nc.scalar.dma_start(out=t, in_=ow[P * pr:P * (pr + 1), :])
            ow_sb.append(t)
        btri_sb = consts.tile([P, P], F32, name="btri_sb")
        nc.scalar.dma_start(out=btri_sb, in_=btri)
        ones4 = consts.tile([P, HEADS], F32, name="ones4")
        nc.vector.memset(ones4, 1.0)

        QT = [qkv.tile([P, S], BF16, name=f"QT{pr}") for pr in range(2)]
        KT = [qkv.tile([P, S], BF16, name=f"KT{pr}") for pr in range(2)]
        CT = [qkv.tile([P, S], BF16, name=f"CT{pr}") for pr in range(2)]
        VA = [qkv.tile([P, HEADS, HD + 1], BF16, name=f"VA{t}") for t in range(NKT)]

        pending_outproj = []

        def emit_outproj(qc):
            for mq in range(QCW // P):
                out_sb = outp.tile([P, D], BF16, name="out_sb")
                q0 = QCW * qc + P * mq
                for ne in range(2):
                    o_ps = spool.tile([P, 2, QCW], F32, name="s_ps")[:, 0, :]
                    for pr2 in range(2):
                        nc.tensor.matmul(
                            o_ps,
                            lhsT=CT[pr2][:, q0:q0 + P],
                            rhs=ow_sb[pr2][:, QCW * ne:QCW * (ne + 1)],
                            start=(pr2 == 0), stop=(pr2 == 1))
                    nc.vector.tensor_copy(out_sb[:, QCW * ne:QCW * (ne + 1)], o_ps)
                nc.gpsimd.dma_start(out=out[q0:q0 + P, :], in_=out_sb)

        def load_pieces(xap, n):
            """8 [128, 1024] pieces of x.T covering the q columns of stages
            n and n+1, plus the bias ones-row piece."""
            ps = []
            for kc in range(KC):
                xt = xpool.tile([P, 2 * QCW], BF16, name="xt")
                nc.sync.dma_start(
                    out=xt,
                    in_=xap[P * kc:P * (kc + 1), QCW * n:QCW * (n + 2)])
                ps.append(xt)
            aug = None
            if has_bias:
                aug = small.tile([1, 2 * QCW], BF16, name="xaug", bufs=3)
                nc.sync.dma_start(
                    out=aug, in_=xap[D:D + 1, QCW * n:QCW * (n + 2)])
            return ps, aug

        xh = {}
        for n in range(NQC):
            # ---- stage n projections: q/k columns + v rows [512n, 512n+512) ----
            if n % 2 == 0:
                xh["q"] = load_pieces(xq, n)
                xh["k"] = load_pieces(xk, n)
                xh["v"] = load_pieces(xv, n)
            hs = (n % 2) * QCW  # column offset within the 2-stage piece
            for key, w_sb, w_aug, dest in (("q", wq_sb, wq_aug, QT),
                                           ("k", wk_sb, wk_aug, KT)):
                x_p, x_a = xh[key]
                for m in range(2):
                    ps = spool.tile([P, 2, QCW], F32, name="s_ps")
                    for kc in range(KC):
                        nc.tensor.matmul(
                            ps[:, 0, :],
                            lhsT=w_sb[kc][:, P * m:P * (m + 1)],
                            rhs=x_p[kc][:, hs:hs + QCW],
                            start=(kc == 0),
                            stop=(not has_bias and kc == KC - 1))
                    if has_bias:
                        nc.tensor.matmul(
                            ps[:, 0, :],
                            lhsT=w_aug[0:1, P * m:P * (m + 1)],
                            rhs=x_a[0:1, hs:hs + QCW],
                            start=False, stop=True)
                    nc.vector.tensor_copy(
                        dest[m][:, QCW * n:QCW * (n + 1)], ps[:, 0, :])
            xv_p, xv_a = xh["v"]
            for mv in range(4):
                m = 4 * n + mv
                ps = spool.tile([P, 2, QCW], F32, name="s_ps")
                for kc in range(KC):
                    nc.tensor.matmul(
                        ps[:, 0, 0:M_DIM],
                        lhsT=xv_p[kc][:, hs + P * mv:hs + P * (mv + 1)],
                        rhs=wv_sb[kc],
                        start=(kc == 0),
                        stop=(not has_bias and kc == KC - 1))
                if has_bias:
                    nc.tensor.matmul(
                        ps[:, 0, 0:M_DIM],
                        lhsT=xv_a[0:1, hs + P * mv:hs + P * (mv + 1)],
                        rhs=wv_aug,
                        start=False, stop=True)
                nc.vector.tensor_copy(
                    VA[m][:, :, 0:HD],
                    ps[:, 0, 0:M_DIM].rearrange("p (h d) -> p h d", h=HEADS))
                nc.vector.tensor_copy(
                    VA[m][:, :, HD:HD + 1],
                    ones4.rearrange("p (h o) -> p h o", o=1))
            if pending_outproj:
                emit_outproj(pending_outproj.pop(0))
            if probes and n == NQC - 1:
                nc.sync.dma_start(out=prb["p_qt"].bitcast(BF16)[:, 0:S], in_=QT[0])
                nc.sync.dma_start(out=prb["p_kt"].bitcast(BF16)[:, 0:S], in_=KT[0])
                nc.sync.dma_start(
                    out=prb["p_va"].bitcast(BF16)[:, 0:HEADS * (HD + 1)],
                    in_=VA[0].rearrange("p h d -> p (h d)"))

            # ---- stage n attention (q chunk n) ----
            qc = n
            for pr in range(2):
                nt = 4 * qc + 4 if mode == "causal" else NKT
                ctxs = [cpool.tile([HD + 1, QCW], F32, name="ctx_ps")
                        for _ in range(2)]
                queues = ([], [])

                def flush_ctx(j):
                    t0, p0, o0 = queues[j].pop(0)
                    nc.tensor.matmul(
                        ctxs[j][:, o0:],
                        lhsT=VA[t0][:, 2 * pr + j, :],
                        rhs=p0[:, j, o0:],
                        start=(t0 == 0), stop=(t0 == nt - 1),
                        skip_group_check=True)

                for t in range(nt):
                    o = max(0, P * t - QCW * qc) if mode == "causal" else 0
                    s_ps = spool.tile([P, 2, QCW], F32, name="s_ps")
                    for j in range(2):
                        nc.tensor.matmul(
                            s_ps[:, j, o:],
                            lhsT=KT[pr][HD * j:HD * (j + 1), P * t:P * (t + 1)],
                            rhs=QT[pr][HD * j:HD * (j + 1),
                                       QCW * qc + o:QCW * (qc + 1)],
                            start=True, stop=True,
                            tile_position=(HD * j, 0))
                    if mode == "causal" and t >= 4 * qc:
                        nc.vector.tensor_add(
                            s_ps[:, :, o:o + P],
                            s_ps[:, :, o:o + P],
                            btri_sb.rearrange("p (a q) -> p a q", a=1)
                            .to_broadcast([P, 2, P]))
                    elif mode == "generic":
                        bt = bpool.tile([P, QCW], F32, name="bt")
                        nc.sync.dma_start(
                            out=bt,
                            in_=bfull[P * t:P * (t + 1), QCW * qc:QCW * (qc + 1)])
                        nc.vector.tensor_add(
                            s_ps, s_ps,
                            bt.rearrange("p (a q) -> p a q", a=1)
                            .to_broadcast([P, 2, QCW]))
                    p_sb = ppool.tile([P, 2, QCW], BF16, name="p_sb")
                    nc.scalar.activation(
                        p_sb[:, :, o:], s_ps[:, :, o:], EXPF, scale=0.125)
                    for j in range(2):
                        queues[j].append((t, p_sb, o))
                    for j in range(2):
                        if len(queues[j]) > 2:
                            flush_ctx(j)
                for j in range(2):
                    while queues[j]:
                        flush_ctx(j)
                for j in range(2):
                    ctx_ps = ctxs[j]
                    l_sb = small.tile([1, QCW], F32, name="l_sb", bufs=3)
                    nc.vector.tensor_copy(l_sb, ctx_ps[HD:HD + 1, :])
                    r_sb = small.tile([1, QCW], F32, name="r_sb", bufs=3)
                    nc.vector.reciprocal_approx_fast(out=r_sb, in_=l_sb)
                    rbc = ppool.tile([HD, QCW], F32, name="rbc", bufs=2)
                    nc.gpsimd.partition_broadcast(out_ap=rbc, in_ap=r_sb)
                    nc.vector.tensor_mul(
                        CT[pr][HD * j:HD * (j + 1), QCW * qc:QCW * (qc + 1)],
                        ctx_ps[0:HD, :], rbc)

            pending_outproj.append(qc)
        emit_outproj(pending_outproj.pop(0))
        if probes:
            nc.sync.dma_start(out=prb["p_ct"].bitcast(BF16)[:, 0:S], in_=CT[0])

    if compile_:
        nc.compile()
    return nc


def _get_nc(mode, has_bias):
    key = (mode, has_bias)
    if key not in _NC_CACHE:
        _NC_CACHE[key] = build_nc(mode, has_bias=has_bias)
    return _NC_CACHE[key]


def _tri_bias():
    g = np.arange(P, dtype=np.int64)
    return np.where(g[None, :] < g[:, None], np.float32(NEG), np.float32(0.0))


def host_prep(query, key, value, attn_mask, q_w, q_b, k_w, k_b, v_w, v_b, o_w, o_b):
    """Build (mode, in_maps) for the 8 cores."""
    mask = np.asarray(attn_mask).astype(bool)
    if np.array_equal(mask, np.triu(np.ones((S, S), bool), 1)):
        mode = "causal"
    elif not mask.any():
        mode = "nomask"
    else:
        mode = "generic"

    import ml_dtypes
    bf16 = ml_dtypes.bfloat16
    ones_row = np.ones((1, S), bf16)

    def prep_x(x):
        return np.vstack([np.ascontiguousarray(x.T).astype(bf16), ones_row])

    xs = {}
    for b in range(2):
        xs[b] = (prep_x(np.asarray(query)[b]), prep_x(np.asarray(key)[b]),
                 prep_x(np.asarray(value)[b]))

    tri = _tri_bias()
    biasT = None
    if mode == "generic":
        biasT = np.ascontiguousarray(
            np.where(mask, np.float32(NEG), np.float32(0.0)).T)

    def prep_w(w, bvec, sl):
        return np.vstack([
            np.ascontiguousarray(np.asarray(w)[sl].T).astype(bf16),
            np.asarray(bvec)[sl][None, :].astype(bf16)])

    in_maps = []
    for c in range(8):
        b, g = divmod(c, 4)
        sl = slice(M_DIM * g, M_DIM * (g + 1))
        m = {
            "xqT": xs[b][0], "xkT": xs[b][1], "xvT": xs[b][2],
            "wqT": prep_w(q_w, q_b, sl),
            "wkT": prep_w(k_w, k_b, sl),
            "wvT": prep_w(v_w, v_b, sl),
            "owT": np.ascontiguousarray(np.asarray(o_w)[:, sl].T).astype(bf16),
            "btri": tri,
        }
        if mode == "generic":
            m["biasT"] = biasT
        in_maps.append(m)
    return mode, in_maps


def kernel(**inputs) -> np.ndarray:
    global LAST_RESULTS
    from concourse.bass_utils import run_bass_kernel_spmd

    mode, in_maps = host_prep(**inputs)
    has_bias = any(
        np.asarray(inputs[k]).any() for k in ("q_b", "k_b", "v_b"))
    nc = _get_nc(mode, has_bias)
    res = run_bass_kernel_spmd(nc, in_maps, core_ids=list(range(8)), trace=TRACE)
    LAST_RESULTS = res
    parts = [np.asarray(res.results[c]["out"]).astype(np.float32)
             for c in range(8)]
    o_b = np.asarray(inputs["o_b"]).astype(np.float32)
    out = np.stack([
        parts[0] + parts[1] + parts[2] + parts[3],
        parts[4] + parts[5] + parts[6] + parts[7],
    ], axis=0) + o_b[None, None, :]
    return out.astype(np.float32)


# revision 14
# speedup vs baseline: 1.1817x; 1.0245x over previous
"""Multi-head attention (B=2, S=2048, D=1024, H=16, causal mask) on 8 TRN2 cores.

Sharding: core c handles batch b = c//4 and 4 heads g = c%4 (dims 256g..256g+256
of the projection space).  Each core computes a partial output [S, D] (its 4
heads' contribution to the out-projection); the host sums the 4 partials per
batch and adds the output bias.

Device layout (per core) keeps the sequence axis on the SBUF free dimension:
  QT, KT  [256, 2048]  (head-dim on partitions, 2 head-pairs of 128)
  V_aug   16 tiles [128, 4, 65]  (seq on partitions; per head 64 dims + ones col)
  scores  S.T tiles [128 k, 512 q] per head; causal blocks above diagonal skipped
  exp     ScalarE, scale=1/8, mask folded in as a -1e9 bias (one [128,128] tri tile)
  ctx.T   [65, 512] PSUM per (head, q-chunk); row 64 = softmax denominator l
  norm    reciprocal_approx_fast on l, partition_broadcast, DVE multiply
  out     ctxT (4 heads stacked, [256, 2048]) @ o_w slice -> [2048, 1024]

The causal path software-pipelines emission: projection / out-projection matmul
"filler" quanta are woven between attention tiles so the PE never waits for the
ScalarE exp (which otherwise limits the attention phase), and the whole x input
is resident in SBUF (DMA'd up-front in need-ordered 512-column chunks).
"""

import numpy as np
from contextlib import ExitStack

import concourse.bacc as bacc
import concourse.bass as bass
import concourse.tile as tile
from concourse import mybir

P = 128
S = 2048
D = 1024
N_HEADS_TOT = 16
HEADS = 4            # per core
HD = 64
M_DIM = HEADS * HD   # 256
KC = 8               # embed-dim 128-chunks
QCW = 512            # q chunk width
NQC = S // QCW       # 4
NKT = S // P         # 16 k-tiles
F32 = mybir.dt.float32
F32R = mybir.dt.float32r
BF16 = mybir.dt.bfloat16
EXPF = mybir.ActivationFunctionType.Exp
NEG = -1.0e9

TRACE = False
LAST_RESULTS = None
_NC_CACHE = {}


class Doler:
    """Dole filler-generator quanta, in order, between primary steps."""

    def __init__(self, gens):
        self.gens = list(gens)
        self.done = 0

    def pump(self, upto=None, k=None):
        """Advance until `done` >= upto (absolute) or by k quanta."""
        if k is not None:
            upto = self.done + k
        while self.done < upto and self.gens:
            try:
                next(self.gens[0])
                self.done += 1
            except StopIteration:
                self.gens.pop(0)

    def drain(self):
        while self.gens:
            try:
                next(self.gens[0])
                self.done += 1
            except StopIteration:
                self.gens.pop(0)


def build_nc_causal(compile_: bool = True, has_bias: bool = False) -> bass.Bass:
    """Interleaved (software-pipelined) causal-mask build."""
    nc = bacc.Bacc("TRN2", target_bir_lowering=False, debug=False)
    xq = nc.dram_tensor("xqT", [D + 1, S], BF16, kind="ExternalInput").ap()
    xk = nc.dram_tensor("xkT", [D + 1, S], BF16, kind="ExternalInput").ap()
    xv = nc.dram_tensor("xvT", [D + 1, S], BF16, kind="ExternalInput").ap()
    wq = nc.dram_tensor("wqT", [D + 1, M_DIM], BF16, kind="ExternalInput").ap()
    wk = nc.dram_tensor("wkT", [D + 1, M_DIM], BF16, kind="ExternalInput").ap()
    wv = nc.dram_tensor("wvT", [D + 1, M_DIM], BF16, kind="ExternalInput").ap()
    ow = nc.dram_tensor("owT", [M_DIM, D], BF16, kind="ExternalInput").ap()
    btri = nc.dram_tensor("btri", [P, P], F32, kind="ExternalInput").ap()
    out = nc.dram_tensor("out", [S, D], BF16, kind="ExternalOutput").ap()

    with tile.TileContext(nc) as tc, ExitStack() as ctx:
        consts = ctx.enter_context(tc.tile_pool(name="consts", bufs=1))
        xpool = ctx.enter_context(tc.tile_pool(name="xpool", bufs=1))
        qkv = ctx.enter_context(tc.tile_pool(name="qkv", bufs=1))
        ppool = ctx.enter_context(tc.tile_pool(name="ppool", bufs=6))
        rpool = ctx.enter_context(tc.tile_pool(name="rpool", bufs=2))
        small = ctx.enter_context(tc.tile_pool(name="small", bufs=4))
        outp = ctx.enter_context(tc.tile_pool(name="outp", bufs=2))
        scp = ctx.enter_context(tc.tile_pool(name="scp", bufs=2, space="PSUM"))
        pjp = ctx.enter_context(tc.tile_pool(name="pjp", bufs=2, space="PSUM"))
        cxp = ctx.enter_context(tc.tile_pool(name="cxp", bufs=2, space="PSUM"))

        # ---- resident weights + xv on the gpsimd (SWDGE) queue, in need
        #      order, keeping the two HWDGE rings dedicated to xq / xk.
        #      Combined DMAs spread their descriptors over all 16 SDMA
        #      engines, so few big transfers beat many small ones. ----
        wq_all = consts.tile([P, KC, M_DIM], BF16, name="wq_all")
        nc.gpsimd.dma_start(
            out=wq_all, in_=wq[0:D, :].rearrange("(kc p) m -> p kc m", p=P))
        wk_all = consts.tile([P, KC, M_DIM], BF16, name="wk_all")
        nc.gpsimd.dma_start(
            out=wk_all, in_=wk[0:D, :].rearrange("(kc p) m -> p kc m", p=P))
        btri_sb = consts.tile([P, P], F32, name="btri_sb")
        nc.gpsimd.dma_start(out=btri_sb, in_=btri)
        wv_all = consts.tile([P, KC, M_DIM], BF16, name="wv_all")
        nc.gpsimd.dma_start(
            out=wv_all, in_=wv[0:D, :].rearrange("(kc p) m -> p kc m", p=P))
        xq_all = xpool.tile([P, KC, S], BF16, name="xq_all")
        xk_all = xpool.tile([P, KC, S], BF16, name="xk_all")
        xv_all = xpool.tile([P, KC, S], BF16, name="xv_all")
        nc.gpsimd.dma_start(
            out=xv_all[:, :, 0:QCW],
            in_=xv[0:D, 0:QCW].rearrange("(kc p) s -> p kc s", p=P))
        ow_all = consts.tile([P, 2, D], BF16, name="ow_all")
        nc.gpsimd.dma_start(
            out=ow_all, in_=ow[0:M_DIM, :].rearrange("(pr p) e -> p pr e", p=P))
        nc.gpsimd.dma_start(
            out=xv_all[:, :, QCW:2 * QCW],
            in_=xv[0:D, QCW:2 * QCW].rearrange("(kc p) s -> p kc s", p=P))
        nc.gpsimd.dma_start(
            out=xv_all[:, :, 2 * QCW:S],
            in_=xv[0:D, 2 * QCW:S].rearrange("(kc p) s -> p kc s", p=P))
        wq_sb = [wq_all[:, kc, :] for kc in range(KC)]
        wk_sb = [wk_all[:, kc, :] for kc in range(KC)]
        wv_sb = [wv_all[:, kc, :] for kc in range(KC)]
        ow_sb = [ow_all[:, pr, :] for pr in range(2)]
        wq_aug = wk_aug = wv_aug = None
        if has_bias:
            wq_aug = consts.tile([1, M_DIM], BF16, name="wq_aug")
            nc.gpsimd.dma_start(out=wq_aug, in_=wq[D:D + 1, :])
            wk_aug = consts.tile([1, M_DIM], BF16, name="wk_aug")
            nc.gpsimd.dma_start(out=wk_aug, in_=wk[D:D + 1, :])
            wv_aug = consts.tile([1, M_DIM], BF16, name="wv_aug")
            nc.gpsimd.dma_start(out=wv_aug, in_=wv[D:D + 1, :])
        ones4 = consts.tile([P, HEADS], F32, name="ones4")
        nc.vector.memset(ones4, 1.0)

        # ---- resident xq / xk: combined [128, 8kc, cols] DMAs on the HWDGE
        #      rings, stage-0 columns first for a fast ramp ----
        for c0, c1 in ((0, QCW), (QCW, 2 * QCW), (2 * QCW, S)):
            for xall, xap, eng in ((xq_all, xq, nc.sync), (xk_all, xk, nc.scalar)):
                eng.dma_start(
                    out=xall[:, :, c0:c1],
                    in_=xap[0:D, c0:c1].rearrange("(kc p) s -> p kc s", p=P))
        xq_sb = [xq_all[:, kc, :] for kc in range(KC)]
        xk_sb = [xk_all[:, kc, :] for kc in range(KC)]
        xv_sb = [xv_all[:, kc, :] for kc in range(KC)]
        xq_aug = xk_aug = xv_aug = None
        if has_bias:
            xq_aug = small.tile([1, S], BF16, name="xq_aug")
            nc.sync.dma_start(out=xq_aug, in_=xq[D:D + 1, :])
            xk_aug = small.tile([1, S], BF16, name="xk_aug")
            nc.sync.dma_start(out=xk_aug, in_=xk[D:D + 1, :])
            xv_aug = small.tile([1, S], BF16, name="xv_aug")
            nc.sync.dma_start(out=xv_aug, in_=xv[D:D + 1, :])

        QT = [qkv.tile([P, S], BF16, name=f"QT{pr}") for pr in range(2)]
        KT = [qkv.tile([P, S], BF16, name=f"KT{pr}") for pr in range(2)]
        CT = [qkv.tile([P, S], BF16, name=f"CT{pr}") for pr in range(2)]
        VA = [qkv.tile([P, HEADS, HD + 1], BF16, name=f"VA{t}") for t in range(NKT)]

        # ---- filler generators (each yield ~= 0.5-0.9us of PE work) ----
        def gen_qkproj(n, w_sb, w_aug, x_sb, x_aug, dest, m):
            c0, c1 = QCW * n, QCW * (n + 1)
            ps = pjp.tile([P, QCW], F32, name="pj_ps")
            for kc in range(KC):
                nc.tensor.matmul(
                    ps,
                    lhsT=w_sb[kc][:, P * m:P * (m + 1)],
                    rhs=x_sb[kc][:, c0:c1],
                    start=(kc == 0),
                    stop=(not has_bias and kc == KC - 1),
                    skip_group_check=True)
                if kc == 3:
                    yield
            if has_bias:
                nc.tensor.matmul(
                    ps,
                    lhsT=w_aug[0:1, P * m:P * (m + 1)],
                    rhs=x_aug[0:1, c0:c1],
                    start=False, stop=True, skip_group_check=True)
            nc.vector.tensor_copy(dest[m][:, c0:c1], ps)
            yield

        def gen_vproj(n, mv):
            hs = QCW * n + P * mv
            ps = pjp.tile([P, QCW], F32, name="pj_ps")
            for kc in range(KC):
                nc.tensor.matmul(
                    ps[:, 0:M_DIM],
                    lhsT=xv_sb[kc][:, hs:hs + P],
                    rhs=wv_sb[kc],
                    start=(kc == 0),
                    stop=(not has_bias and kc == KC - 1),
                    skip_group_check=True)
                if kc == 3:
                    yield
            if has_bias:
                nc.tensor.matmul(
                    ps[:, 0:M_DIM],
                    lhsT=xv_aug[0:1, hs:hs + P],
                    rhs=wv_aug,
                    start=False, stop=True, skip_group_check=True)
            m = 4 * n + mv
            nc.vector.tensor_copy(
                VA[m][:, :, 0:HD],
                ps[:, 0:M_DIM].rearrange("p (h d) -> p h d", h=HEADS))
            nc.vector.tensor_copy(
                VA[m][:, :, HD:HD + 1],
                ones4.rearrange("p (h o) -> p h o", o=1))
            yield

        def gen_outproj(qc, mq):
            q0 = QCW * qc + P * mq
            out_sb = outp.tile([P, D], BF16, name="out_sb")
            for ne in range(2):
                o_ps = pjp.tile([P, QCW], F32, name="pj_ps")
                for pr2 in range(2):
                    nc.tensor.matmul(
                        o_ps,
                        lhsT=CT[pr2][:, q0:q0 + P],
                        rhs=ow_sb[pr2][:, QCW * ne:QCW * (ne + 1)],
                        start=(pr2 == 0), stop=(pr2 == 1))
                nc.vector.tensor_copy(out_sb[:, QCW * ne:QCW * (ne + 1)], o_ps)
                if ne == 1:
                    nc.sync.dma_start(out=out[q0:q0 + P, :], in_=out_sb)
                yield

        # ---- attention block for one (qc, pr): yields per tile / misc step ----
        def gen_attn(qc, pr, doler, needs):
            nt = 4 * qc + 4
            ctxs = [cxp.tile([HD + 1, QCW], F32, name="ctx_ps")
                    for _ in range(2)]
            queue = []

            def flush():
                t0, p0, o0 = queue.pop(0)
                for j in range(2):
                    nc.tensor.matmul(
                        ctxs[j][:, o0:],
                        lhsT=VA[t0][:, 2 * pr + j, :],
                        rhs=p0[:, j, o0:],
                        start=(t0 == 0), stop=(t0 == nt - 1),
                        skip_group_check=True)

            nfill = needs.get("total", 0)
            base = doler.done
            for t in range(nt):
                # lazy spread: hold ~6 quanta back for the drain/normalize
                # phase so the PE keeps filler work at block boundaries
                spread = base + (nfill * (t + 1)) // (nt + 6)
                doler.pump(upto=max(spread, base + needs.get(t, 0)))
                o = max(0, P * t - QCW * qc)
                s_ps = scp.tile([P, 2, QCW], F32, name="s_ps")
                for j in range(2):
                    nc.tensor.matmul(
                        s_ps[:, j, o:],
                        lhsT=KT[pr][HD * j:HD * (j + 1), P * t:P * (t + 1)],
                        rhs=QT[pr][HD * j:HD * (j + 1),
                                   QCW * qc + o:QCW * (qc + 1)],
                        start=True, stop=True,
                        tile_position=(HD * j, 0))
                if t >= 4 * qc:
                    nc.vector.tensor_add(
                        s_ps[:, :, o:o + P],
                        s_ps[:, :, o:o + P],
                        btri_sb.rearrange("p (a q) -> p a q", a=1)
                        .to_broadcast([P, 2, P]))
                p_sb = ppool.tile([P, 2, QCW], BF16, name="p_sb")
                nc.scalar.activation(
                    p_sb[:, :, o:], s_ps[:, :, o:], EXPF, scale=0.125)
                queue.append((t, p_sb, o))
                if len(queue) > 2:
                    flush()
            while queue:
                doler.pump(k=2)
                flush()
            # normalize both head halves, chains pipelined across engines
            l_sbs, r_sbs, rbcs = [], [], []
            for j in range(2):
                l_sb = small.tile([1, QCW], F32, name="l_sb", bufs=3)
                nc.vector.tensor_copy(l_sb, ctxs[j][HD:HD + 1, :])
                l_sbs.append(l_sb)
            doler.pump(k=2)
            for j in range(2):
                r_sb = small.tile([1, QCW], F32, name="r_sb", bufs=3)
                nc.vector.reciprocal_approx_fast(out=r_sb, in_=l_sbs[j])
                r_sbs.append(r_sb)
            for j in range(2):
                rbc = rpool.tile([HD, QCW], F32, name="rbc")
                nc.gpsimd.partition_broadcast(out_ap=rbc, in_ap=r_sbs[j])
                rbcs.append(rbc)
            doler.pump(k=2)
            for j in range(2):
                nc.vector.tensor_mul(
                    CT[pr][HD * j:HD * (j + 1), QCW * qc:QCW * (qc + 1)],
                    ctxs[j][0:HD, :], rbcs[j])

        # ---- stage loop ----
        for n in range(NQC):
            qc = n
            # pr0 block: fillers = qproj m0, kproj m0, vproj 0-3, qproj m1
            f0 = Doler([
                gen_qkproj(n, wq_sb, wq_aug, xq_sb, xq_aug, QT, 0),
                gen_qkproj(n, wk_sb, wk_aug, xk_sb, xk_aug, KT, 0),
                gen_vproj(n, 0), gen_vproj(n, 1),
                gen_vproj(n, 2), gen_vproj(n, 3),
                gen_qkproj(n, wq_sb, wq_aug, xq_sb, xq_aug, QT, 1),
            ])
            # minimum cumulative quanta before tile t (absolute indices)
            needs0 = {0: 2, 4 * n: 6, 4 * n + 1: 8, 4 * n + 2: 10,
                      4 * n + 3: 12, "total": 14}
            gen_attn(qc, 0, f0, needs0)
            f0.drain()

            # pr1 block: fillers = kproj m1, outproj(qc-1)
            gens1 = [gen_qkproj(n, wk_sb, wk_aug, xk_sb, xk_aug, KT, 1)]
            if n >= 1:
                gens1 += [gen_outproj(n - 1, mq) for mq in range(4)]
            f1 = Doler(gens1)
            needs1 = {4 * n: 2, "total": 2 + (8 if n >= 1 else 0)}
            gen_attn(qc, 1, f1, needs1)
            f1.drain()

        for mq in range(4):
            for _ in gen_outproj(NQC - 1, mq):
                pass

    if compile_:
        nc.compile()
    return nc


def build_nc(mode: str, compile_: bool = True, probes: bool = False,
             has_bias: bool = False) -> bass.Bass:
    """mode in {causal, nomask, generic}; causal uses the pipelined build."""
    if mode == "causal" and not probes:
        return build_nc_causal(compile_=compile_, has_bias=has_bias)
    nc = bacc.Bacc("TRN2", target_bir_lowering=False, debug=False)
    prb = {}
    if probes:
        for nm, shape in (("p_qt", [P, S]), ("p_kt", [P, S]),
                          ("p_va", [P, HEADS * (HD + 1)]), ("p_ct", [P, S])):
            prb[nm] = nc.dram_tensor(nm, shape, F32, kind="ExternalOutput").ap()
    xq = nc.dram_tensor("xqT", [D + 1, S], BF16, kind="ExternalInput").ap()
    xk = nc.dram_tensor("xkT", [D + 1, S], BF16, kind="ExternalInput").ap()
    xv = nc.dram_tensor("xvT", [D + 1, S], BF16, kind="ExternalInput").ap()
    wq = nc.dram_tensor("wqT", [D + 1, M_DIM], BF16, kind="ExternalInput").ap()
    wk = nc.dram_tensor("wkT", [D + 1, M_DIM], BF16, kind="ExternalInput").ap()
    wv = nc.dram_tensor("wvT", [D + 1, M_DIM], BF16, kind="ExternalInput").ap()
    ow = nc.dram_tensor("owT", [M_DIM, D], BF16, kind="ExternalInput").ap()
    btri = nc.dram_tensor("btri", [P, P], F32, kind="ExternalInput").ap()
    bfull = None
    if mode == "generic":
        bfull = nc.dram_tensor("biasT", [S, S], F32, kind="ExternalInput").ap()
    out = nc.dram_tensor("out", [S, D], BF16, kind="ExternalOutput").ap()

    with tile.TileContext(nc) as tc, ExitStack() as ctx:
        consts = ctx.enter_context(tc.tile_pool(name="consts", bufs=1))
        xpool = ctx.enter_context(tc.tile_pool(name="xpool", bufs=26))
        qkv = ctx.enter_context(tc.tile_pool(name="qkv", bufs=1))
        ppool = ctx.enter_context(tc.tile_pool(name="ppool", bufs=6))
        bpool = ctx.enter_context(tc.tile_pool(name="bpool", bufs=2))
        small = ctx.enter_context(tc.tile_pool(name="small", bufs=4))
        outp = ctx.enter_context(tc.tile_pool(name="outp", bufs=2))
        spool = ctx.enter_context(tc.tile_pool(name="spsum", bufs=3, space="PSUM"))
        cpool = ctx.enter_context(tc.tile_pool(name="cpsum", bufs=2, space="PSUM"))

        # ---- resident weights ----
        def load_w(ap_dram, nm):
            tiles = []
            for kc in range(KC):
                t = consts.tile([P, M_DIM], BF16, name=f"{nm}{kc}")
                nc.scalar.dma_start(out=t, in_=ap_dram[P * kc:P * (kc + 1), :])
                tiles.append(t)
            aug = None
            if has_bias:
                aug = consts.tile([1, M_DIM], BF16, name=f"{nm}_aug")
                nc.sync.dma_start(out=aug, in_=ap_dram[D:D + 1, :])
            return tiles, aug

        wq_sb, wq_aug = load_w(wq, "wq")
        wk_sb, wk_aug = load_w(wk, "wk")
        wv_sb, wv_aug = load_w(wv, "wv")
        ow_sb = []
        for pr in range(2):
            t = consts.tile([P, D], BF16, name=f"ow{pr}")
            nc.scalar.dma_start(out=t, in_=ow[P * pr:P * (pr + 1), :])
            ow_sb.append(t)
        btri_sb = consts.tile([P, P], F32, name="btri_sb")
        nc.scalar.dma_start(out=btri_sb, in_=btri)
        ones4 = consts.tile([P, HEADS], F32, name="ones4")
        nc.vector.memset(ones4, 1.0)

        QT = [qkv.tile([P, S], BF16, name=f"QT{pr}") for pr in range(2)]
        KT = [qkv.tile([P, S], BF16, name=f"KT{pr}") for pr in range(2)]
        CT = [qkv.tile([P, S], BF16, name=f"CT{pr}") for pr in range(2)]
        VA = [qkv.tile([P, HEADS, HD + 1], BF16, name=f"VA{t}") for t in range(NKT)]

        pending_outproj = []

        def emit_outproj(qc):
            for mq in range(QCW // P):
                out_sb = outp.tile([P, D], BF16, name="out_sb")
                q0 = QCW * qc + P * mq
                for ne in range(2):
                    o_ps = spool.tile([P, 2, QCW], F32, name="s_ps")[:, 0, :]
                    for pr2 in range(2):
                        nc.tensor.matmul(
                            o_ps,
                            lhsT=CT[pr2][:, q0:q0 + P],
                            rhs=ow_sb[pr2][:, QCW * ne:QCW * (ne + 1)],
                            start=(pr2 == 0), stop=(pr2 == 1))
                    nc.vector.tensor_copy(out_sb[:, QCW * ne:QCW * (ne + 1)], o_ps)
                nc.gpsimd.dma_start(out=out[q0:q0 + P, :], in_=out_sb)

        def load_pieces(xap, n):
            """8 [128, 1024] pieces of x.T covering the q columns of stages
            n and n+1, plus the bias ones-row piece."""
            ps = []
            for kc in range(KC):
                xt = xpool.tile([P, 2 * QCW], BF16, name="xt")
                nc.sync.dma_start(
                    out=xt,
                    in_=xap[P * kc:P * (kc + 1), QCW * n:QCW * (n + 2)])
                ps.append(xt)
            aug = None
            if has_bias:
                aug = small.tile([1, 2 * QCW], BF16, name="xaug", bufs=3)
                nc.sync.dma_start(
                    out=aug, in_=xap[D:D + 1, QCW * n:QCW * (n + 2)])
            return ps, aug

        xh = {}
        for n in range(NQC):
            # ---- stage n projections: q/k columns + v rows [512n, 512n+512) ----
            if n % 2 == 0:
                xh["q"] = load_pieces(xq, n)
                xh["k"] = load_pieces(xk, n)
                xh["v"] = load_pieces(xv, n)
            hs = (n % 2) * QCW  # column offset within the 2-stage piece
            for key, w_sb, w_aug, dest in (("q", wq_sb, wq_aug, QT),
                                           ("k", wk_sb, wk_aug, KT)):
                x_p, x_a = xh[key]
                for m in range(2):
                    ps = spool.tile([P, 2, QCW], F32, name="s_ps")
                    for kc in range(KC):
                        nc.tensor.matmul(
                            ps[:, 0, :],
                            lhsT=w_sb[kc][:, P * m:P * (m + 1)],
                            rhs=x_p[kc][:, hs:hs + QCW],
                            start=(kc == 0),
                            stop=(not has_bias and kc == KC - 1))
                    if has_bias:
                        nc.tensor.matmul(
                            ps[:, 0, :],
                            lhsT=w_aug[0:1, P * m:P * (m + 1)],
                            rhs=x_a[0:1, hs:hs + QCW],
                            start=False, stop=True)
                    nc.vector.tensor_copy(
                        dest[m][:, QCW * n:QCW * (n + 1)], ps[:, 0, :])
            xv_p, xv_a = xh["v"]
            for mv in range(4):
                m = 4 * n + mv
                ps = spool.tile([P, 2, QCW], F32, name="s_ps")
                for kc in range(KC):
                    nc.tensor.matmul(
                        ps[:, 0, 0:M_DIM],
                        lhsT=xv_p[kc][:, hs + P * mv:hs + P * (mv + 1)],
                        rhs=wv_sb[kc],
                        start=(kc == 0),
                        stop=(not has_bias and kc == KC - 1))
                if has_bias:
                    nc.tensor.matmul(
                        ps[:, 0, 0:M_DIM],
                        lhsT=xv_a[0:1, hs + P * mv:hs + P * (mv + 1)],
                        rhs=wv_aug,
                        start=False, stop=True)
                nc.vector.tensor_copy(
                    VA[m][:, :, 0:HD],
                    ps[:, 0, 0:M_DIM].rearrange("p (h d) -> p h d", h=HEADS))
                nc.vector.tensor_copy(
                    VA[m][:, :, HD:HD + 1],
                    ones4.rearrange("p (h o) -> p h o", o=1))
            if pending_outproj:
                emit_outproj(pending_outproj.pop(0))
            if probes and n == NQC - 1:
                nc.sync.dma_start(out=prb["p_qt"].bitcast(BF16)[:, 0:S], in_=QT[0])
                nc.sync.dma_start(out=prb["p_kt"].bitcast(BF16)[:, 0:S], in_=KT[0])
                nc.sync.dma_start(
                    out=prb["p_va"].bitcast(BF16)[:, 0:HEADS * (HD + 1)],
                    in_=VA[0].rearrange("p h d -> p (h d)"))

            # ---- stage n attention (q chunk n) ----
            qc = n
            for pr in range(2):
                nt = 4 * qc + 4 if mode == "causal" else NKT
                ctxs = [cpool.tile([HD + 1, QCW], F32, name="ctx_ps")
                        for _ in range(2)]
                queues = ([], [])

                def flush_ctx(j):
                    t0, p0, o0 = queues[j].pop(0)
                    nc.tensor.matmul(
                        ctxs[j][:, o0:],
                        lhsT=VA[t0][:, 2 * pr + j, :],
                        rhs=p0[:, j, o0:],
                        start=(t0 == 0), stop=(t0 == nt - 1),
                        skip_group_check=True)

                for t in range(nt):
                    o = max(0, P * t - QCW * qc) if mode == "causal" else 0
                    s_ps = spool.tile([P, 2, QCW], F32, name="s_ps")
                    for j in range(2):
                        nc.tensor.matmul(
                            s_ps[:, j, o:],
                            lhsT=KT[pr][HD * j:HD * (j + 1), P * t:P * (t + 1)],
                            rhs=QT[pr][HD * j:HD * (j + 1),
                                       QCW * qc + o:QCW * (qc + 1)],
                            start=True, stop=True,
                            tile_position=(HD * j, 0))
                    if mode == "causal" and t >= 4 * qc:
                        nc.vector.tensor_add(
                            s_ps[:, :, o:o + P],
                            s_ps[:, :, o:o + P],
                            btri_sb.rearrange("p (a q) -> p a q", a=1)
                            .to_broadcast([P, 2, P]))
                    elif mode == "generic":
                        bt = bpool.tile([P, QCW], F32, name="bt")
                        nc.sync.dma_start(
                            out=bt,
                            in_=bfull[P * t:P * (t + 1), QCW * qc:QCW * (qc + 1)])
                        nc.vector.tensor_add(
                            s_ps, s_ps,
                            bt.rearrange("p (a q) -> p a q", a=1)
                            .to_broadcast([P, 2, QCW]))
                    p_sb = ppool.tile([P, 2, QCW], BF16, name="p_sb")
                    nc.scalar.activation(
                        p_sb[:, :, o:], s_ps[:, :, o:], EXPF, scale=0.125)
                    for j in range(2):
                        queues[j].append((t, p_sb, o))
                    for j in range(2):
                        if len(queues[j]) > 2:
                            flush_ctx(j)
                for j in range(2):
                    while queues[j]:
                        flush_ctx(j)
                for j in range(2):
                    ctx_ps = ctxs[j]
                    l_sb = small.tile([1, QCW], F32, name="l_sb", bufs=3)
                    nc.vector.tensor_copy(l_sb, ctx_ps[HD:HD + 1, :])
                    r_sb = small.tile([1, QCW], F32, name="r_sb", bufs=3)
                    nc.vector.reciprocal_approx_fast(out=r_sb, in_=l_sb)
                    rbc = ppool.tile([HD, QCW], F32, name="rbc", bufs=2)
                    nc.gpsimd.partition_broadcast(out_ap=rbc, in_ap=r_sb)
                    nc.vector.tensor_mul(
                        CT[pr][HD * j:HD * (j + 1), QCW * qc:QCW * (qc + 1)],
                        ctx_ps[0:HD, :], rbc)

            pending_outproj.append(qc)
        emit_outproj(pending_outproj.pop(0))
        if probes:
            nc.sync.dma_start(out=prb["p_ct"].bitcast(BF16)[:, 0:S], in_=CT[0])

    if compile_:
        nc.compile()
    return nc


def _get_nc(mode, has_bias):
    key = (mode, has_bias)
    if key not in _NC_CACHE:
        _NC_CACHE[key] = build_nc(mode, has_bias=has_bias)
    return _NC_CACHE[key]


def _tri_bias():
    g = np.arange(P, dtype=np.int64)
    return np.where(g[None, :] < g[:, None], np.float32(NEG), np.float32(0.0))


def host_prep(query, key, value, attn_mask, q_w, q_b, k_w, k_b, v_w, v_b, o_w, o_b):
    """Build (mode, in_maps) for the 8 cores."""
    mask = np.asarray(attn_mask).astype(bool)
    if np.array_equal(mask, np.triu(np.ones((S, S), bool), 1)):
        mode = "causal"
    elif not mask.any():
        mode = "nomask"
    else:
        mode = "generic"

    import ml_dtypes
    bf16 = ml_dtypes.bfloat16
    ones_row = np.ones((1, S), bf16)

    def prep_x(x):
        return np.vstack([np.ascontiguousarray(x.T).astype(bf16), ones_row])

    xs = {}
    for b in range(2):
        xs[b] = (prep_x(np.asarray(query)[b]), prep_x(np.asarray(key)[b]),
                 prep_x(np.asarray(value)[b]))

    tri = _tri_bias()
    biasT = None
    if mode == "generic":
        biasT = np.ascontiguousarray(
            np.where(mask, np.float32(NEG), np.float32(0.0)).T)

    def prep_w(w, bvec, sl):
        return np.vstack([
            np.ascontiguousarray(np.asarray(w)[sl].T).astype(bf16),
            np.asarray(bvec)[sl][None, :].astype(bf16)])

    in_maps = []
    for c in range(8):
        b, g = divmod(c, 4)
        sl = slice(M_DIM * g, M_DIM * (g + 1))
        m = {
            "xqT": xs[b][0], "xkT": xs[b][1], "xvT": xs[b][2],
            "wqT": prep_w(q_w, q_b, sl),
            "wkT": prep_w(k_w, k_b, sl),
            "wvT": prep_w(v_w, v_b, sl),
            "owT": np.ascontiguousarray(np.asarray(o_w)[:, sl].T).astype(bf16),
            "btri": tri,
        }
        if mode == "generic":
            m["biasT"] = biasT
        in_maps.append(m)
    return mode, in_maps


def kernel(**inputs) -> np.ndarray:
    global LAST_RESULTS
    from concourse.bass_utils import run_bass_kernel_spmd

    mode, in_maps = host_prep(**inputs)
    has_bias = any(
        np.asarray(inputs[k]).any() for k in ("q_b", "k_b", "v_b"))
    nc = _get_nc(mode, has_bias)
    res = run_bass_kernel_spmd(nc, in_maps, core_ids=list(range(8)), trace=TRACE)
    LAST_RESULTS = res
    parts = [np.asarray(res.results[c]["out"]).astype(np.float32)
             for c in range(8)]
    o_b = np.asarray(inputs["o_b"]).astype(np.float32)
    out = np.stack([
        parts[0] + parts[1] + parts[2] + parts[3],
        parts[4] + parts[5] + parts[6] + parts[7],
    ], axis=0) + o_b[None, None, :]
    return out.astype(np.float32)


# revision 19
# speedup vs baseline: 1.2642x; 1.0698x over previous
"""Multi-head attention (B=2, S=2048, D=1024, H=16, causal mask) on 8 TRN2 cores.

Sharding: core c handles batch b = c//4 and 4 heads g = c%4 (dims 256g..256g+256
of the projection space).  Each core computes a partial output [S, D] (its 4
heads' contribution to the out-projection); the host sums the 4 partials per
batch and adds the output bias.

Device layout (per core) keeps the sequence axis on the SBUF free dimension:
  QT, KT  [256, 2048]  (head-dim on partitions, 2 head-pairs of 128)
  V_aug   16 tiles [128, 4, 65]  (seq on partitions; per head 64 dims + ones col)
  scores  S.T tiles [128 k, 512 q] per head; causal blocks above diagonal skipped
  exp     ScalarE, scale=1/8, mask folded in as a -1e9 bias (one [128,128] tri tile)
  ctx.T   [65, 512] PSUM per (head, q-chunk); row 64 = softmax denominator l
  norm    reciprocal_approx_fast on l, partition_broadcast, DVE multiply
  out     ctxT (4 heads stacked, [256, 2048]) @ o_w slice -> [2048, 1024]

The causal path software-pipelines emission: projection / out-projection matmul
"filler" quanta are woven between attention tiles so the PE never waits for the
ScalarE exp (which otherwise limits the attention phase), and the whole x input
is resident in SBUF (DMA'd up-front in need-ordered 512-column chunks).
"""

import numpy as np
from contextlib import ExitStack

import concourse.bacc as bacc
import concourse.bass as bass
import concourse.tile as tile
from concourse import mybir

P = 128
S = 2048
D = 1024
N_HEADS_TOT = 16
HEADS = 4            # per core
HD = 64
M_DIM = HEADS * HD   # 256
KC = 8               # embed-dim 128-chunks
QCW = 512            # q chunk width
NQC = S // QCW       # 4
NKT = S // P         # 16 k-tiles
F32 = mybir.dt.float32
F32R = mybir.dt.float32r
BF16 = mybir.dt.bfloat16
EXPF = mybir.ActivationFunctionType.Exp
NEG = -1.0e9

TRACE = False
LAST_RESULTS = None
_NC_CACHE = {}


class Doler:
    """Dole filler-generator quanta, in order, between primary steps."""

    def __init__(self, gens):
        self.gens = list(gens)
        self.done = 0

    def pump(self, upto=None, k=None):
        """Advance until `done` >= upto (absolute) or by k quanta."""
        if k is not None:
            upto = self.done + k
        while self.done < upto and self.gens:
            try:
                next(self.gens[0])
                self.done += 1
            except StopIteration:
                self.gens.pop(0)

    def drain(self):
        while self.gens:
            try:
                next(self.gens[0])
                self.done += 1
            except StopIteration:
                self.gens.pop(0)


def build_nc_causal(compile_: bool = True, has_bias: bool = False) -> bass.Bass:
    """Interleaved (software-pipelined) causal-mask build."""
    nc = bacc.Bacc("TRN2", target_bir_lowering=False, debug=False)
    xq = nc.dram_tensor("xqT", [D + 1, S], BF16, kind="ExternalInput").ap()
    xk = nc.dram_tensor("xkT", [D + 1, S], BF16, kind="ExternalInput").ap()
    xv = nc.dram_tensor("xvT", [D + 1, S], BF16, kind="ExternalInput").ap()
    wq = nc.dram_tensor("wqT", [D + 1, M_DIM], BF16, kind="ExternalInput").ap()
    wk = nc.dram_tensor("wkT", [D + 1, M_DIM], BF16, kind="ExternalInput").ap()
    wv = nc.dram_tensor("wvT", [D + 1, M_DIM], BF16, kind="ExternalInput").ap()
    ow = nc.dram_tensor("owT", [M_DIM, D], BF16, kind="ExternalInput").ap()
    btri = nc.dram_tensor("btri", [P, P], F32, kind="ExternalInput").ap()
    out = nc.dram_tensor("out", [S, D], BF16, kind="ExternalOutput").ap()

    with tile.TileContext(nc) as tc, ExitStack() as ctx:
        consts = ctx.enter_context(tc.tile_pool(name="consts", bufs=1))
        xpool = ctx.enter_context(tc.tile_pool(name="xpool", bufs=1))
        qkv = ctx.enter_context(tc.tile_pool(name="qkv", bufs=1))
        ppool = ctx.enter_context(tc.tile_pool(name="ppool", bufs=6))
        rpool = ctx.enter_context(tc.tile_pool(name="rpool", bufs=2))
        small = ctx.enter_context(tc.tile_pool(name="small", bufs=4))
        outp = ctx.enter_context(tc.tile_pool(name="outp", bufs=2))
        scp = ctx.enter_context(tc.tile_pool(name="scp", bufs=2, space="PSUM"))
        pjp = ctx.enter_context(tc.tile_pool(name="pjp", bufs=2, space="PSUM"))
        cxp = ctx.enter_context(tc.tile_pool(name="cxp", bufs=2, space="PSUM"))

        # ---- resident weights + xv on the gpsimd (SWDGE) queue, in need
        #      order, keeping the two HWDGE rings dedicated to xq / xk.
        #      Combined DMAs spread their descriptors over all 16 SDMA
        #      engines, so few big transfers beat many small ones. ----
        wq_all = consts.tile([P, KC, M_DIM], BF16, name="wq_all")
        nc.gpsimd.dma_start(
            out=wq_all, in_=wq[0:D, :].rearrange("(kc p) m -> p kc m", p=P))
        wk_all = consts.tile([P, KC, M_DIM], BF16, name="wk_all")
        nc.gpsimd.dma_start(
            out=wk_all, in_=wk[0:D, :].rearrange("(kc p) m -> p kc m", p=P))
        btri_sb = consts.tile([P, P], F32, name="btri_sb")
        nc.gpsimd.dma_start(out=btri_sb, in_=btri)
        wv_all = consts.tile([P, KC, M_DIM], BF16, name="wv_all")
        nc.gpsimd.dma_start(
            out=wv_all, in_=wv[0:D, :].rearrange("(kc p) m -> p kc m", p=P))
        xq_all = xpool.tile([P, KC, S], BF16, name="xq_all")
        xk_all = xpool.tile([P, KC, S], BF16, name="xk_all")
        xv_all = xpool.tile([P, KC, S], BF16, name="xv_all")
        nc.gpsimd.dma_start(
            out=xv_all[:, :, 0:QCW],
            in_=xv[0:D, 0:QCW].rearrange("(kc p) s -> p kc s", p=P))
        ow_all = consts.tile([P, 2, D], BF16, name="ow_all")

        def load_x_chunk(n):
            """Emit stage-n x column-chunk loads on the gpsimd ring.  Called
            mid-schedule: the gpsimd sequencer reaches these dma_starts only
            after the preceding (semaphore-waiting) partition_broadcasts, so
            late chunks don't steal SDMA-engine share from earlier ones."""
            c0, c1 = QCW * n, QCW * (n + 1)
            for xall, xap in ((xq_all, xq), (xk_all, xk), (xv_all, xv)):
                nc.gpsimd.dma_start(
                    out=xall[:, :, c0:c1],
                    in_=xap[0:D, c0:c1].rearrange("(kc p) s -> p kc s", p=P))

        def load_ow():
            nc.gpsimd.dma_start(
                out=ow_all,
                in_=ow[0:M_DIM, :].rearrange("(pr p) e -> p pr e", p=P))
        wq_sb = [wq_all[:, kc, :] for kc in range(KC)]
        wk_sb = [wk_all[:, kc, :] for kc in range(KC)]
        wv_sb = [wv_all[:, kc, :] for kc in range(KC)]
        ow_sb = [ow_all[:, pr, :] for pr in range(2)]
        wq_aug = wk_aug = wv_aug = None
        if has_bias:
            wq_aug = consts.tile([1, M_DIM], BF16, name="wq_aug")
            nc.gpsimd.dma_start(out=wq_aug, in_=wq[D:D + 1, :])
            wk_aug = consts.tile([1, M_DIM], BF16, name="wk_aug")
            nc.gpsimd.dma_start(out=wk_aug, in_=wk[D:D + 1, :])
            wv_aug = consts.tile([1, M_DIM], BF16, name="wv_aug")
            nc.gpsimd.dma_start(out=wv_aug, in_=wv[D:D + 1, :])
        ones4 = consts.tile([P, HEADS], F32, name="ones4")
        nc.vector.memset(ones4, 1.0)

        # ---- stage-0 xq / xk chunks on the HWDGE rings (fast ramp); later
        #      chunks are emitted at gate points inside the stage loop ----
        nc.sync.dma_start(
            out=xq_all[:, :, 0:QCW],
            in_=xq[0:D, 0:QCW].rearrange("(kc p) s -> p kc s", p=P))
        nc.scalar.dma_start(
            out=xk_all[:, :, 0:QCW],
            in_=xk[0:D, 0:QCW].rearrange("(kc p) s -> p kc s", p=P))
        xq_sb = [xq_all[:, kc, :] for kc in range(KC)]
        xk_sb = [xk_all[:, kc, :] for kc in range(KC)]
        xv_sb = [xv_all[:, kc, :] for kc in range(KC)]
        xq_aug = xk_aug = xv_aug = None
        if has_bias:
            xq_aug = small.tile([1, S], BF16, name="xq_aug")
            nc.sync.dma_start(out=xq_aug, in_=xq[D:D + 1, :])
            xk_aug = small.tile([1, S], BF16, name="xk_aug")
            nc.sync.dma_start(out=xk_aug, in_=xk[D:D + 1, :])
            xv_aug = small.tile([1, S], BF16, name="xv_aug")
            nc.sync.dma_start(out=xv_aug, in_=xv[D:D + 1, :])

        QT = [qkv.tile([P, S], BF16, name=f"QT{pr}") for pr in range(2)]
        KT = [qkv.tile([P, S], BF16, name=f"KT{pr}") for pr in range(2)]
        CT = [qkv.tile([P, S], BF16, name=f"CT{pr}") for pr in range(2)]
        VA = [qkv.tile([P, HEADS, HD + 1], BF16, name=f"VA{t}") for t in range(NKT)]

        # ---- filler generators (each yield ~= 0.5-0.9us of PE work) ----
        def gen_qkproj(n, w_sb, w_aug, x_sb, x_aug, dest, m):
            c0, c1 = QCW * n, QCW * (n + 1)
            ps = pjp.tile([P, QCW], F32, name="pj_ps")
            for kc in range(KC):
                nc.tensor.matmul(
                    ps,
                    lhsT=w_sb[kc][:, P * m:P * (m + 1)],
                    rhs=x_sb[kc][:, c0:c1],
                    start=(kc == 0),
                    stop=(not has_bias and kc == KC - 1),
                    skip_group_check=True)
                if kc == 3:
                    yield
            if has_bias:
                nc.tensor.matmul(
                    ps,
                    lhsT=w_aug[0:1, P * m:P * (m + 1)],
                    rhs=x_aug[0:1, c0:c1],
                    start=False, stop=True, skip_group_check=True)
            nc.vector.tensor_copy(dest[m][:, c0:c1], ps)
            yield

        def gen_vproj(n, mv):
            hs = QCW * n + P * mv
            ps = pjp.tile([P, QCW], F32, name="pj_ps")
            for kc in range(KC):
                nc.tensor.matmul(
                    ps[:, 0:M_DIM],
                    lhsT=xv_sb[kc][:, hs:hs + P],
                    rhs=wv_sb[kc],
                    start=(kc == 0),
                    stop=(not has_bias and kc == KC - 1),
                    skip_group_check=True)
                if kc == 3:
                    yield
            if has_bias:
                nc.tensor.matmul(
                    ps[:, 0:M_DIM],
                    lhsT=xv_aug[0:1, hs:hs + P],
                    rhs=wv_aug,
                    start=False, stop=True, skip_group_check=True)
            m = 4 * n + mv
            nc.vector.tensor_copy(
                VA[m][:, :, 0:HD],
                ps[:, 0:M_DIM].rearrange("p (h d) -> p h d", h=HEADS))
            nc.vector.tensor_copy(
                VA[m][:, :, HD:HD + 1],
                ones4.rearrange("p (h o) -> p h o", o=1))
            yield

        def gen_outproj(qc, mq):
            q0 = QCW * qc + P * mq
            out_sb = outp.tile([P, D], BF16, name="out_sb")
            for ne in range(2):
                o_ps = pjp.tile([P, QCW], F32, name="pj_ps")
                for pr2 in range(2):
                    nc.tensor.matmul(
                        o_ps,
                        lhsT=CT[pr2][:, q0:q0 + P],
                        rhs=ow_sb[pr2][:, QCW * ne:QCW * (ne + 1)],
                        start=(pr2 == 0), stop=(pr2 == 1))
                nc.vector.tensor_copy(out_sb[:, QCW * ne:QCW * (ne + 1)], o_ps)
                if ne == 1:
                    nc.sync.dma_start(out=out[q0:q0 + P, :], in_=out_sb)
                yield

        # ---- attention block for one (qc, pr): yields per tile / misc step ----
        def gen_attn(qc, pr, doler, needs):
            nt = 4 * qc + 4
            ctxs = [cxp.tile([HD + 1, QCW], F32, name="ctx_ps")
                    for _ in range(2)]
            queue = []

            def flush():
                t0, p0, o0 = queue.pop(0)
                for j in range(2):
                    nc.tensor.matmul(
                        ctxs[j][:, o0:],
                        lhsT=VA[t0][:, 2 * pr + j, :],
                        rhs=p0[:, j, o0:],
                        start=(t0 == 0), stop=(t0 == nt - 1),
                        skip_group_check=True)

            nfill = needs.get("total", 0)
            base = doler.done
            for t in range(nt):
                # lazy spread: hold quanta back for the drain/normalize
                # phase so the PE keeps filler work at block boundaries
                spread = base + (nfill * (t + 1)) // (nt + needs.get("hold", 6))
                doler.pump(upto=max(spread, base + needs.get(t, 0)))
                o = max(0, P * t - QCW * qc)
                s_ps = scp.tile([P, 2, QCW], F32, name="s_ps")
                for j in range(2):
                    nc.tensor.matmul(
                        s_ps[:, j, o:],
                        lhsT=KT[pr][HD * j:HD * (j + 1), P * t:P * (t + 1)],
                        rhs=QT[pr][HD * j:HD * (j + 1),
                                   QCW * qc + o:QCW * (qc + 1)],
                        start=True, stop=True,
                        tile_position=(HD * j, 0))
                if t >= 4 * qc:
                    nc.vector.tensor_add(
                        s_ps[:, :, o:o + P],
                        s_ps[:, :, o:o + P],
                        btri_sb.rearrange("p (a q) -> p a q", a=1)
                        .to_broadcast([P, 2, P]))
                p_sb = ppool.tile([P, 2, QCW], BF16, name="p_sb")
                nc.scalar.activation(
                    p_sb[:, :, o:], s_ps[:, :, o:], EXPF, scale=0.125)
                queue.append((t, p_sb, o))
                if len(queue) > 2:
                    flush()
            while queue:
                doler.pump(k=2)
                flush()
            # normalize both head halves, chains pipelined across engines
            l_sbs, r_sbs, rbcs = [], [], []
            for j in range(2):
                l_sb = small.tile([1, QCW], F32, name="l_sb", bufs=3)
                nc.scalar.activation(
                    l_sb, ctxs[j][HD:HD + 1, :],
                    mybir.ActivationFunctionType.Copy)
                l_sbs.append(l_sb)
            doler.pump(k=2)
            for j in range(2):
                r_sb = small.tile([1, QCW], F32, name="r_sb", bufs=3)
                nc.vector.reciprocal_approx_fast(out=r_sb, in_=l_sbs[j])
                r_sbs.append(r_sb)
            for j in range(2):
                rbc = rpool.tile([HD, QCW], F32, name="rbc")
                nc.gpsimd.partition_broadcast(out_ap=rbc, in_ap=r_sbs[j])
                rbcs.append(rbc)
            doler.pump(k=2)
            for j in range(2):
                nc.vector.tensor_mul(
                    CT[pr][HD * j:HD * (j + 1), QCW * qc:QCW * (qc + 1)],
                    ctxs[j][0:HD, :], rbcs[j])

        # ---- stage loop ----
        for n in range(NQC):
            qc = n
            # pr0 block: fillers = qproj m0, kproj m0, vproj 0-3, qproj m1
            f0 = Doler([
                gen_qkproj(n, wq_sb, wq_aug, xq_sb, xq_aug, QT, 0),
                gen_qkproj(n, wk_sb, wk_aug, xk_sb, xk_aug, KT, 0),
                gen_vproj(n, 0), gen_vproj(n, 1),
                gen_vproj(n, 2), gen_vproj(n, 3),
                gen_qkproj(n, wq_sb, wq_aug, xq_sb, xq_aug, QT, 1),
            ])
            # minimum cumulative quanta before tile t (absolute indices)
            needs0 = {0: 2, 4 * n: 6, 4 * n + 1: 8, 4 * n + 2: 10,
                      4 * n + 3: 12, "total": 14}
            gen_attn(qc, 0, f0, needs0)
            f0.drain()
            if n == 0:
                load_x_chunk(1)
            elif n == 1:
                load_x_chunk(3)

            # pr1 block: fillers = kproj m1, outproj(qc-1)
            gens1 = [gen_qkproj(n, wk_sb, wk_aug, xk_sb, xk_aug, KT, 1)]
            if n >= 1:
                gens1 += [gen_outproj(n - 1, mq) for mq in range(4)]
            f1 = Doler(gens1)
            needs1 = {4 * n: 2, "total": 2 + (8 if n >= 1 else 0)}
            if n == NQC - 1:
                needs1["hold"] = 12
            gen_attn(qc, 1, f1, needs1)
            f1.drain()
            if n == 0:
                load_ow()
                load_x_chunk(2)

        for mq in range(4):
            for _ in gen_outproj(NQC - 1, mq):
                pass

    if compile_:
        nc.compile()
    return nc


def build_nc(mode: str, compile_: bool = True, probes: bool = False,
             has_bias: bool = False) -> bass.Bass:
    """mode in {causal, nomask, generic}; causal uses the pipelined build."""
    if mode == "causal" and not probes:
        return build_nc_causal(compile_=compile_, has_bias=has_bias)
    nc = bacc.Bacc("TRN2", target_bir_lowering=False, debug=False)
    prb = {}
    if probes:
        for nm, shape in (("p_qt", [P, S]), ("p_kt", [P, S]),
                          ("p_va", [P, HEADS * (HD + 1)]), ("p_ct", [P, S])):
            prb[nm] = nc.dram_tensor(nm, shape, F32, kind="ExternalOutput").ap()
    xq = nc.dram_tensor("xqT", [D + 1, S], BF16, kind="ExternalInput").ap()
    xk = nc.dram_tensor("xkT", [D + 1, S], BF16, kind="ExternalInput").ap()
    xv = nc.dram_tensor("xvT", [D + 1, S], BF16, kind="ExternalInput").ap()
    wq = nc.dram_tensor("wqT", [D + 1, M_DIM], BF16, kind="ExternalInput").ap()
    wk = nc.dram_tensor("wkT", [D + 1, M_DIM], BF16, kind="ExternalInput").ap()
    wv = nc.dram_tensor("wvT", [D + 1, M_DIM], BF16, kind="ExternalInput").ap()
    ow = nc.dram_tensor("owT", [M_DIM, D], BF16, kind="ExternalInput").ap()
    btri = nc.dram_tensor("btri", [P, P], F32, kind="ExternalInput").ap()
    bfull = None
    if mode == "generic":
        bfull = nc.dram_tensor("biasT", [S, S], F32, kind="ExternalInput").ap()
    out = nc.dram_tensor("out", [S, D], BF16, kind="ExternalOutput").ap()

    with tile.TileContext(nc) as tc, ExitStack() as ctx:
        consts = ctx.enter_context(tc.tile_pool(name="consts", bufs=1))
        xpool = ctx.enter_context(tc.tile_pool(name="xpool", bufs=26))
        qkv = ctx.enter_context(tc.tile_pool(name="qkv", bufs=1))
        ppool = ctx.enter_context(tc.tile_pool(name="ppool", bufs=6))
        bpool = ctx.enter_context(tc.tile_pool(name="bpool", bufs=2))
        small = ctx.enter_context(tc.tile_pool(name="small", bufs=4))
        outp = ctx.enter_context(tc.tile_pool(name="outp", bufs=2))
        spool = ctx.enter_context(tc.tile_pool(name="spsum", bufs=3, space="PSUM"))
        cpool = ctx.enter_context(tc.tile_pool(name="cpsum", bufs=2, space="PSUM"))

        # ---- resident weights ----
        def load_w(ap_dram, nm):
            tiles = []
            for kc in range(KC):
                t = consts.tile([P, M_DIM], BF16, name=f"{nm}{kc}")
                nc.scalar.dma_start(out=t, in_=ap_dram[P * kc:P * (kc + 1), :])
                tiles.append(t)
            aug = None
            if has_bias:
                aug = consts.tile([1, M_DIM], BF16, name=f"{nm}_aug")
                nc.sync.dma_start(out=aug, in_=ap_dram[D:D + 1, :])
            return tiles, aug

        wq_sb, wq_aug = load_w(wq, "wq")
        wk_sb, wk_aug = load_w(wk, "wk")
        wv_sb, wv_aug = load_w(wv, "wv")
        ow_sb = []
        for pr in range(2):
            t = consts.tile([P, D], BF16, name=f"ow{pr}")
            nc.scalar.dma_start(out=t, in_=ow[P * pr:P * (pr + 1), :])
            ow_sb.append(t)
        btri_sb = consts.tile([P, P], F32, name="btri_sb")
        nc.scalar.dma_start(out=btri_sb, in_=btri)
        ones4 = consts.tile([P, HEADS], F32, name="ones4")
        nc.vector.memset(ones4, 1.0)

        QT = [qkv.tile([P, S], BF16, name=f"QT{pr}") for pr in range(2)]
        KT = [qkv.tile([P, S], BF16, name=f"KT{pr}") for pr in range(2)]
        CT = [qkv.tile([P, S], BF16, name=f"CT{pr}") for pr in range(2)]
        VA = [qkv.tile([P, HEADS, HD + 1], BF16, name=f"VA{t}") for t in range(NKT)]

        pending_outproj = []

        def emit_outproj(qc):
            for mq in range(QCW // P):
                out_sb = outp.tile([P, D], BF16, name="out_sb")
                q0 = QCW * qc + P * mq
                for ne in range(2):
                    o_ps = spool.tile([P, 2, QCW], F32, name="s_ps")[:, 0, :]
                    for pr2 in range(2):
                        nc.tensor.matmul(
                            o_ps,
                            lhsT=CT[pr2][:, q0:q0 + P],
                            rhs=ow_sb[pr2][:, QCW * ne:QCW * (ne + 1)],
                            start=(pr2 == 0), stop=(pr2 == 1))
                    nc.vector.tensor_copy(out_sb[:, QCW * ne:QCW * (ne + 1)], o_ps)
                nc.gpsimd.dma_start(out=out[q0:q0 + P, :], in_=out_sb)

        def load_pieces(xap, n):
            """8 [128, 1024] pieces of x.T covering the q columns of stages
            n and n+1, plus the bias ones-row piece."""
            ps = []
            for kc in range(KC):
                xt = xpool.tile([P, 2 * QCW], BF16, name="xt")
                nc.sync.dma_start(
                    out=xt,
                    in_=xap[P * kc:P * (kc + 1), QCW * n:QCW * (n + 2)])
                ps.append(xt)
            aug = None
            if has_bias:
                aug = small.tile([1, 2 * QCW], BF16, name="xaug", bufs=3)
                nc.sync.dma_start(
                    out=aug, in_=xap[D:D + 1, QCW * n:QCW * (n + 2)])
            return ps, aug

        xh = {}
        for n in range(NQC):
            # ---- stage n projections: q/k columns + v rows [512n, 512n+512) ----
            if n % 2 == 0:
                xh["q"] = load_pieces(xq, n)
                xh["k"] = load_pieces(xk, n)
                xh["v"] = load_pieces(xv, n)
            hs = (n % 2) * QCW  # column offset within the 2-stage piece
            for key, w_sb, w_aug, dest in (("q", wq_sb, wq_aug, QT),
                                           ("k", wk_sb, wk_aug, KT)):
                x_p, x_a = xh[key]
                for m in range(2):
                    ps = spool.tile([P, 2, QCW], F32, name="s_ps")
                    for kc in range(KC):
                        nc.tensor.matmul(
                            ps[:, 0, :],
                            lhsT=w_sb[kc][:, P * m:P * (m + 1)],
                            rhs=x_p[kc][:, hs:hs + QCW],
                            start=(kc == 0),
                            stop=(not has_bias and kc == KC - 1))
                    if has_bias:
                        nc.tensor.matmul(
                            ps[:, 0, :],
                            lhsT=w_aug[0:1, P * m:P * (m + 1)],
                            rhs=x_a[0:1, hs:hs + QCW],
                            start=False, stop=True)
                    nc.vector.tensor_copy(
                        dest[m][:, QCW * n:QCW * (n + 1)], ps[:, 0, :])
            xv_p, xv_a = xh["v"]
            for mv in range(4):
                m = 4 * n + mv
                ps = spool.tile([P, 2, QCW], F32, name="s_ps")
                for kc in range(KC):
                    nc.tensor.matmul(
                        ps[:, 0, 0:M_DIM],
                        lhsT=xv_p[kc][:, hs + P * mv:hs + P * (mv + 1)],
                        rhs=wv_sb[kc],
                        start=(kc == 0),
                        stop=(not has_bias and kc == KC - 1))
                if has_bias:
                    nc.tensor.matmul(
                        ps[:, 0, 0:M_DIM],
                        lhsT=xv_a[0:1, hs + P * mv:hs + P * (mv + 1)],
                        rhs=wv_aug,
                        start=False, stop=True)
                nc.vector.tensor_copy(
                    VA[m][:, :, 0:HD],
                    ps[:, 0, 0:M_DIM].rearrange("p (h d) -> p h d", h=HEADS))
                nc.vector.tensor_copy(
                    VA[m][:, :, HD:HD + 1],
                    ones4.rearrange("p (h o) -> p h o", o=1))
            if pending_outproj:
                emit_outproj(pending_outproj.pop(0))
            if probes and n == NQC - 1:
                nc.sync.dma_start(out=prb["p_qt"].bitcast(BF16)[:, 0:S], in_=QT[0])
                nc.sync.dma_start(out=prb["p_kt"].bitcast(BF16)[:, 0:S], in_=KT[0])
                nc.sync.dma_start(
                    out=prb["p_va"].bitcast(BF16)[:, 0:HEADS * (HD + 1)],
                    in_=VA[0].rearrange("p h d -> p (h d)"))

            # ---- stage n attention (q chunk n) ----
            qc = n
            for pr in range(2):
                nt = 4 * qc + 4 if mode == "causal" else NKT
                ctxs = [cpool.tile([HD + 1, QCW], F32, name="ctx_ps")
                        for _ in range(2)]
                queues = ([], [])

                def flush_ctx(j):
                    t0, p0, o0 = queues[j].pop(0)
                    nc.tensor.matmul(
                        ctxs[j][:, o0:],
                        lhsT=VA[t0][:, 2 * pr + j, :],
                        rhs=p0[:, j, o0:],
                        start=(t0 == 0), stop=(t0 == nt - 1),
                        skip_group_check=True)

                for t in range(nt):
                    o = max(0, P * t - QCW * qc) if mode == "causal" else 0
                    s_ps = spool.tile([P, 2, QCW], F32, name="s_ps")
                    for j in range(2):
                        nc.tensor.matmul(
                            s_ps[:, j, o:],
                            lhsT=KT[pr][HD * j:HD * (j + 1), P * t:P * (t + 1)],
                            rhs=QT[pr][HD * j:HD * (j + 1),
                                       QCW * qc + o:QCW * (qc + 1)],
                            start=True, stop=True,
                            tile_position=(HD * j, 0))
                    if mode == "causal" and t >= 4 * qc:
                        nc.vector.tensor_add(
                            s_ps[:, :, o:o + P],
                            s_ps[:, :, o:o + P],
                            btri_sb.rearrange("p (a q) -> p a q", a=1)
                            .to_broadcast([P, 2, P]))
                    elif mode == "generic":
                        bt = bpool.tile([P, QCW], F32, name="bt")
                        nc.sync.dma_start(
                            out=bt,
                            in_=bfull[P * t:P * (t + 1), QCW * qc:QCW * (qc + 1)])
                        nc.vector.tensor_add(
                            s_ps, s_ps,
                            bt.rearrange("p (a q) -> p a q", a=1)
                            .to_broadcast([P, 2, QCW]))
                    p_sb = ppool.tile([P, 2, QCW], BF16, name="p_sb")
                    nc.scalar.activation(
                        p_sb[:, :, o:], s_ps[:, :, o:], EXPF, scale=0.125)
                    for j in range(2):
                        queues[j].append((t, p_sb, o))
                    for j in range(2):
                        if len(queues[j]) > 2:
                            flush_ctx(j)
                for j in range(2):
                    while queues[j]:
                        flush_ctx(j)
                for j in range(2):
                    ctx_ps = ctxs[j]
                    l_sb = small.tile([1, QCW], F32, name="l_sb", bufs=3)
                    nc.vector.tensor_copy(l_sb, ctx_ps[HD:HD + 1, :])
                    r_sb = small.tile([1, QCW], F32, name="r_sb", bufs=3)
                    nc.vector.reciprocal_approx_fast(out=r_sb, in_=l_sb)
                    rbc = ppool.tile([HD, QCW], F32, name="rbc", bufs=2)
                    nc.gpsimd.partition_broadcast(out_ap=rbc, in_ap=r_sb)
                    nc.vector.tensor_mul(
                        CT[pr][HD * j:HD * (j + 1), QCW * qc:QCW * (qc + 1)],
                        ctx_ps[0:HD, :], rbc)

            pending_outproj.append(qc)
        emit_outproj(pending_outproj.pop(0))
        if probes:
            nc.sync.dma_start(out=prb["p_ct"].bitcast(BF16)[:, 0:S], in_=CT[0])

    if compile_:
        nc.compile()
    return nc


def _get_nc(mode, has_bias):
    key = (mode, has_bias)
    if key not in _NC_CACHE:
        _NC_CACHE[key] = build_nc(mode, has_bias=has_bias)
    return _NC_CACHE[key]


def _tri_bias():
    g = np.arange(P, dtype=np.int64)
    return np.where(g[None, :] < g[:, None], np.float32(NEG), np.float32(0.0))


def host_prep(query, key, value, attn_mask, q_w, q_b, k_w, k_b, v_w, v_b, o_w, o_b):
    """Build (mode, in_maps) for the 8 cores."""
    mask = np.asarray(attn_mask).astype(bool)
    if np.array_equal(mask, np.triu(np.ones((S, S), bool), 1)):
        mode = "causal"
    elif not mask.any():
        mode = "nomask"
    else:
        mode = "generic"

    import ml_dtypes
    bf16 = ml_dtypes.bfloat16
    ones_row = np.ones((1, S), bf16)

    def prep_x(x):
        return np.vstack([np.ascontiguousarray(x.T).astype(bf16), ones_row])

    xs = {}
    for b in range(2):
        xs[b] = (prep_x(np.asarray(query)[b]), prep_x(np.asarray(key)[b]),
                 prep_x(np.asarray(value)[b]))

    tri = _tri_bias()
    biasT = None
    if mode == "generic":
        biasT = np.ascontiguousarray(
            np.where(mask, np.float32(NEG), np.float32(0.0)).T)

    def prep_w(w, bvec, sl):
        return np.vstack([
            np.ascontiguousarray(np.asarray(w)[sl].T).astype(bf16),
            np.asarray(bvec)[sl][None, :].astype(bf16)])

    in_maps = []
    for c in range(8):
        b, g = divmod(c, 4)
        sl = slice(M_DIM * g, M_DIM * (g + 1))
        m = {
            "xqT": xs[b][0], "xkT": xs[b][1], "xvT": xs[b][2],
            "wqT": prep_w(q_w, q_b, sl),
            "wkT": prep_w(k_w, k_b, sl),
            "wvT": prep_w(v_w, v_b, sl),
            "owT": np.ascontiguousarray(np.asarray(o_w)[:, sl].T).astype(bf16),
            "btri": tri,
        }
        if mode == "generic":
            m["biasT"] = biasT
        in_maps.append(m)
    return mode, in_maps


def kernel(**inputs) -> np.ndarray:
    global LAST_RESULTS
    from concourse.bass_utils import run_bass_kernel_spmd

    mode, in_maps = host_prep(**inputs)
    has_bias = any(
        np.asarray(inputs[k]).any() for k in ("q_b", "k_b", "v_b"))
    nc = _get_nc(mode, has_bias)
    res = run_bass_kernel_spmd(nc, in_maps, core_ids=list(range(8)), trace=TRACE)
    LAST_RESULTS = res
    parts = [np.asarray(res.results[c]["out"]).astype(np.float32)
             for c in range(8)]
    o_b = np.asarray(inputs["o_b"]).astype(np.float32)
    out = np.stack([
        parts[0] + parts[1] + parts[2] + parts[3],
        parts[4] + parts[5] + parts[6] + parts[7],
    ], axis=0) + o_b[None, None, :]
    return out.astype(np.float32)


# revision 30
# speedup vs baseline: 1.2784x; 1.0112x over previous
"""Multi-head attention (B=2, S=2048, D=1024, H=16, causal mask) on 8 TRN2 cores.

Sharding: core c handles batch b = c//4 and 4 heads g = c%4 (dims 256g..256g+256
of the projection space).  Each core computes a partial output [S, D] (its 4
heads' contribution to the out-projection); the host sums the 4 partials per
batch and adds the output bias.

Device layout (per core) keeps the sequence axis on the SBUF free dimension:
  QT, KT  [256, 2048]  (head-dim on partitions, 2 head-pairs of 128)
  V_aug   16 tiles [128, 4, 65]  (seq on partitions; per head 64 dims + ones col)
  scores  S.T tiles [128 k, 512 q] per head; causal blocks above diagonal skipped
  exp     ScalarE, scale=1/8, mask folded in as a -1e9 bias (one [128,128] tri tile)
  ctx.T   [65, 512] PSUM per (head, q-chunk); row 64 = softmax denominator l
  norm    reciprocal_approx_fast on l, partition_broadcast, DVE multiply
  out     ctxT (4 heads stacked, [256, 2048]) @ o_w slice -> [2048, 1024]

The causal path software-pipelines emission: projection / out-projection matmul
"filler" quanta are woven between attention tiles so the PE never waits for the
ScalarE exp (which otherwise limits the attention phase), and the whole x input
is resident in SBUF (DMA'd up-front in need-ordered 512-column chunks).
"""

import numpy as np
from contextlib import ExitStack

import concourse.bacc as bacc
import concourse.bass as bass
import concourse.tile as tile
from concourse import mybir

P = 128
S = 2048
D = 1024
N_HEADS_TOT = 16
HEADS = 4            # per core
HD = 64
M_DIM = HEADS * HD   # 256
KC = 8               # embed-dim 128-chunks
QCW = 512            # q chunk width
NQC = S // QCW       # 4
NKT = S // P         # 16 k-tiles
F32 = mybir.dt.float32
F32R = mybir.dt.float32r
BF16 = mybir.dt.bfloat16
EXPF = mybir.ActivationFunctionType.Exp
NEG = -1.0e9

TRACE = False
LAST_RESULTS = None
_NC_CACHE = {}


class Doler:
    """Dole filler-generator quanta, in order, between primary steps."""

    def __init__(self, gens):
        self.gens = list(gens)
        self.done = 0

    def pump(self, upto=None, k=None):
        """Advance until `done` >= upto (absolute) or by k quanta."""
        if k is not None:
            upto = self.done + k
        while self.done < upto and self.gens:
            try:
                next(self.gens[0])
                self.done += 1
            except StopIteration:
                self.gens.pop(0)

    def drain(self):
        while self.gens:
            try:
                next(self.gens[0])
                self.done += 1
            except StopIteration:
                self.gens.pop(0)


def build_nc_causal(compile_: bool = True, has_bias: bool = False) -> bass.Bass:
    """Interleaved (software-pipelined) causal-mask build."""
    nc = bacc.Bacc("TRN2", target_bir_lowering=False, debug=False)
    xq = nc.dram_tensor("xqT", [D + 1, S], BF16, kind="ExternalInput").ap()
    xk = nc.dram_tensor("xkT", [D + 1, S], BF16, kind="ExternalInput").ap()
    xv = nc.dram_tensor("xvT", [D + 1, S], BF16, kind="ExternalInput").ap()
    wq = nc.dram_tensor("wqT", [D + 1, M_DIM], BF16, kind="ExternalInput").ap()
    wk = nc.dram_tensor("wkT", [D + 1, M_DIM], BF16, kind="ExternalInput").ap()
    wv = nc.dram_tensor("wvT", [D + 1, M_DIM], BF16, kind="ExternalInput").ap()
    ow = nc.dram_tensor("owT", [M_DIM, D], BF16, kind="ExternalInput").ap()
    btri = nc.dram_tensor("btri", [P, P], F32, kind="ExternalInput").ap()
    out = nc.dram_tensor("out", [S, D], BF16, kind="ExternalOutput").ap()

    with tile.TileContext(nc) as tc, ExitStack() as ctx:
        consts = ctx.enter_context(tc.tile_pool(name="consts", bufs=1))
        xpool = ctx.enter_context(tc.tile_pool(name="xpool", bufs=1))
        qkv = ctx.enter_context(tc.tile_pool(name="qkv", bufs=1))
        ppool = ctx.enter_context(tc.tile_pool(name="ppool", bufs=6))
        rpool = ctx.enter_context(tc.tile_pool(name="rpool", bufs=2))
        small = ctx.enter_context(tc.tile_pool(name="small", bufs=4))
        outp = ctx.enter_context(tc.tile_pool(name="outp", bufs=2))
        scp = ctx.enter_context(tc.tile_pool(name="scp", bufs=2, space="PSUM"))
        pjp = ctx.enter_context(tc.tile_pool(name="pjp", bufs=2, space="PSUM"))
        cxp = ctx.enter_context(tc.tile_pool(name="cxp", bufs=2, space="PSUM"))

        # ---- resident weights + xv on the gpsimd (SWDGE) queue, in need
        #      order, keeping the two HWDGE rings dedicated to xq / xk.
        #      Combined DMAs spread their descriptors over all 16 SDMA
        #      engines, so few big transfers beat many small ones. ----
        wq_all = consts.tile([P, KC, M_DIM], BF16, name="wq_all")
        nc.gpsimd.dma_start(
            out=wq_all, in_=wq[0:D, :].rearrange("(kc p) m -> p kc m", p=P))
        wk_all = consts.tile([P, KC, M_DIM], BF16, name="wk_all")
        nc.gpsimd.dma_start(
            out=wk_all, in_=wk[0:D, :].rearrange("(kc p) m -> p kc m", p=P))
        btri_sb = consts.tile([P, P], F32, name="btri_sb")
        nc.gpsimd.dma_start(out=btri_sb, in_=btri)
        wv_all = consts.tile([P, KC, M_DIM], BF16, name="wv_all")
        nc.gpsimd.dma_start(
            out=wv_all, in_=wv[0:D, :].rearrange("(kc p) m -> p kc m", p=P))
        xq_all = xpool.tile([P, KC, S], BF16, name="xq_all")
        xk_all = xpool.tile([P, KC, S], BF16, name="xk_all")
        xv_all = xpool.tile([P, KC, S], BF16, name="xv_all")
        nc.gpsimd.dma_start(
            out=xv_all[:, :, 0:QCW],
            in_=xv[0:D, 0:QCW].rearrange("(kc p) s -> p kc s", p=P))
        ow_all = consts.tile([P, 2, D], BF16, name="ow_all")

        def load_x_chunk(n, keys=None):
            """Emit stage-n x column-chunk loads on the gpsimd ring.  Called
            mid-schedule: the gpsimd sequencer reaches these dma_starts only
            after the preceding (semaphore-waiting) gpsimd ops, so late
            chunks don't steal SDMA-engine share from earlier ones."""
            c0, c1 = QCW * n, QCW * (n + 1)
            sel = {"q": ((xq_all, xq),), "kv": ((xk_all, xk), (xv_all, xv)),
                   None: ((xq_all, xq), (xk_all, xk), (xv_all, xv))}[keys]
            for xall, xap in sel:
                nc.gpsimd.dma_start(
                    out=xall[:, :, c0:c1],
                    in_=xap[0:D, c0:c1].rearrange("(kc p) s -> p kc s", p=P))

        def load_ow():
            nc.gpsimd.dma_start(
                out=ow_all,
                in_=ow[0:M_DIM, :].rearrange("(pr p) e -> p pr e", p=P))
        wq_sb = [wq_all[:, kc, :] for kc in range(KC)]
        wk_sb = [wk_all[:, kc, :] for kc in range(KC)]
        wv_sb = [wv_all[:, kc, :] for kc in range(KC)]
        ow_sb = [ow_all[:, pr, :] for pr in range(2)]
        wq_aug = wk_aug = wv_aug = None
        if has_bias:
            wq_aug = consts.tile([1, M_DIM], BF16, name="wq_aug")
            nc.gpsimd.dma_start(out=wq_aug, in_=wq[D:D + 1, :])
            wk_aug = consts.tile([1, M_DIM], BF16, name="wk_aug")
            nc.gpsimd.dma_start(out=wk_aug, in_=wk[D:D + 1, :])
            wv_aug = consts.tile([1, M_DIM], BF16, name="wv_aug")
            nc.gpsimd.dma_start(out=wv_aug, in_=wv[D:D + 1, :])
        ones4 = consts.tile([P, HEADS], F32, name="ones4")
        nc.vector.memset(ones4, 1.0)
        ones64 = consts.tile([1, HD], BF16, name="ones64")
        nc.vector.memset(ones64, 1.0)

        # ---- stage-0 xq / xk chunks on the HWDGE rings (fast ramp); later
        #      chunks are emitted at gate points inside the stage loop ----
        nc.sync.dma_start(
            out=xq_all[:, :, 0:QCW],
            in_=xq[0:D, 0:QCW].rearrange("(kc p) s -> p kc s", p=P))
        nc.scalar.dma_start(
            out=xk_all[:, :, 0:QCW],
            in_=xk[0:D, 0:QCW].rearrange("(kc p) s -> p kc s", p=P))
        xq_sb = [xq_all[:, kc, :] for kc in range(KC)]
        xk_sb = [xk_all[:, kc, :] for kc in range(KC)]
        xv_sb = [xv_all[:, kc, :] for kc in range(KC)]
        xq_aug = xk_aug = xv_aug = None
        if has_bias:
            xq_aug = small.tile([1, S], BF16, name="xq_aug")
            nc.sync.dma_start(out=xq_aug, in_=xq[D:D + 1, :])
            xk_aug = small.tile([1, S], BF16, name="xk_aug")
            nc.sync.dma_start(out=xk_aug, in_=xk[D:D + 1, :])
            xv_aug = small.tile([1, S], BF16, name="xv_aug")
            nc.sync.dma_start(out=xv_aug, in_=xv[D:D + 1, :])

        QT = [qkv.tile([P, S], BF16, name=f"QT{pr}") for pr in range(2)]
        KT = [qkv.tile([P, S], BF16, name=f"KT{pr}") for pr in range(2)]
        CT = [qkv.tile([P, S], BF16, name=f"CT{pr}") for pr in range(2)]
        VA = [qkv.tile([P, HEADS, HD + 1], BF16, name=f"VA{t}") for t in range(NKT)]

        # ---- filler generators (each yield ~= 0.5-0.9us of PE work) ----
        def gen_qkproj(n, w_sb, w_aug, x_sb, x_aug, dest, m):
            c0, c1 = QCW * n, QCW * (n + 1)
            ps = pjp.tile([P, QCW], F32, name="pj_ps")
            for kc in range(KC):
                nc.tensor.matmul(
                    ps,
                    lhsT=w_sb[kc][:, P * m:P * (m + 1)],
                    rhs=x_sb[kc][:, c0:c1],
                    start=(kc == 0),
                    stop=(not has_bias and kc == KC - 1),
                    skip_group_check=True)
                if kc == 3:
                    yield
            if has_bias:
                nc.tensor.matmul(
                    ps,
                    lhsT=w_aug[0:1, P * m:P * (m + 1)],
                    rhs=x_aug[0:1, c0:c1],
                    start=False, stop=True, skip_group_check=True)
            nc.vector.tensor_copy(dest[m][:, c0:c1], ps)
            yield

        def gen_vproj(n, mv):
            hs = QCW * n + P * mv
            ps = pjp.tile([P, QCW], F32, name="pj_ps")
            for kc in range(KC):
                nc.tensor.matmul(
                    ps[:, 0:M_DIM],
                    lhsT=xv_sb[kc][:, hs:hs + P],
                    rhs=wv_sb[kc],
                    start=(kc == 0),
                    stop=(not has_bias and kc == KC - 1),
                    skip_group_check=True)
                if kc == 3:
                    yield
            if has_bias:
                nc.tensor.matmul(
                    ps[:, 0:M_DIM],
                    lhsT=xv_aug[0:1, hs:hs + P],
                    rhs=wv_aug,
                    start=False, stop=True, skip_group_check=True)
            m = 4 * n + mv
            nc.vector.tensor_copy(
                VA[m][:, :, 0:HD],
                ps[:, 0:M_DIM].rearrange("p (h d) -> p h d", h=HEADS))
            nc.vector.tensor_copy(
                VA[m][:, :, HD:HD + 1],
                ones4.rearrange("p (h o) -> p h o", o=1))
            yield

        def gen_outproj(qc, mq, final=False):
            q0 = QCW * qc + P * mq
            out_sb = outp.tile([P, D], BF16, name="out_sb")
            for ne in range(2):
                o_ps = pjp.tile([P, QCW], F32, name="pj_ps")
                for pr2 in range(2):
                    nc.tensor.matmul(
                        o_ps,
                        lhsT=CT[pr2][:, q0:q0 + P],
                        rhs=ow_sb[pr2][:, QCW * ne:QCW * (ne + 1)],
                        start=(pr2 == 0), stop=(pr2 == 1))
                dst = out_sb[:, QCW * ne:QCW * (ne + 1)]
                if final and ne == 1:
                    # tail: DVE handles ne=0, ACT ne=1 -> casts in parallel
                    nc.scalar.activation(
                        dst, o_ps, mybir.ActivationFunctionType.Copy)
                else:
                    nc.vector.tensor_copy(dst, o_ps)
                if ne == 1:
                    if final:
                        nc.sync.dma_start(
                            out=out[q0:q0 + P, 0:QCW], in_=out_sb[:, 0:QCW])
                        nc.scalar.dma_start(
                            out=out[q0:q0 + P, QCW:D], in_=out_sb[:, QCW:D])
                    else:
                        nc.sync.dma_start(out=out[q0:q0 + P, :], in_=out_sb)
                yield

        # ---- attention block for one (qc, pr): yields per tile / misc step ----
        def gen_attn(qc, pr, doler, needs, hooks=None):
            nt = 4 * qc + 4
            ctxs = [cxp.tile([HD + 1, QCW], F32, name="ctx_ps")
                    for _ in range(2)]
            queue = []

            def flush():
                t0, p0, o0 = queue.pop(0)
                for j in range(2):
                    nc.tensor.matmul(
                        ctxs[j][:, o0:],
                        lhsT=VA[t0][:, 2 * pr + j, :],
                        rhs=p0[:, j, o0:],
                        start=(t0 == 0), stop=(t0 == nt - 1),
                        skip_group_check=True)

            nfill = needs.get("total", 0)
            base = doler.done
            for t in range(nt):
                # lazy spread: hold quanta back for the drain/normalize
                # phase so the PE keeps filler work at block boundaries
                spread = base + (nfill * (t + 1)) // (nt + needs.get("hold", 6))
                doler.pump(upto=max(spread, base + needs.get(t, 0)))
                o = max(0, P * t - QCW * qc)
                s_ps = scp.tile([P, 2, QCW], F32, name="s_ps")
                for j in range(2):
                    nc.tensor.matmul(
                        s_ps[:, j, o:],
                        lhsT=KT[pr][HD * j:HD * (j + 1), P * t:P * (t + 1)],
                        rhs=QT[pr][HD * j:HD * (j + 1),
                                   QCW * qc + o:QCW * (qc + 1)],
                        start=True, stop=True,
                        tile_position=(HD * j, 0))
                if t >= 4 * qc:
                    nc.vector.tensor_add(
                        s_ps[:, :, o:o + P],
                        s_ps[:, :, o:o + P],
                        btri_sb.rearrange("p (a q) -> p a q", a=1)
                        .to_broadcast([P, 2, P]))
                p_sb = ppool.tile([P, 2, QCW], BF16, name="p_sb")
                nc.scalar.activation(
                    p_sb[:, :, o:], s_ps[:, :, o:], EXPF, scale=0.125)
                queue.append((t, p_sb, o))
                if len(queue) > 2:
                    flush()
                if hooks and t in hooks:
                    hooks[t]()
            while queue:
                doler.pump(k=2)
                flush()
            # normalize both head halves, chains pipelined across engines
            l_sbs, r_sbs, rbcs = [], [], []
            for j in range(2):
                l_sb = small.tile([1, QCW], F32, name="l_sb", bufs=3)
                nc.scalar.activation(
                    l_sb, ctxs[j][HD:HD + 1, :],
                    mybir.ActivationFunctionType.Copy)
                l_sbs.append(l_sb)
            doler.pump(k=2)
            for j in range(2):
                r_sb = small.tile([1, QCW], F32, name="r_sb", bufs=3)
                nc.vector.reciprocal_approx_fast(out=r_sb, in_=l_sbs[j])
                r_sbs.append(r_sb)
            if False and qc == NQC - 1 and pr == 1:
                # tail block: broadcast 1/l via a PE outer product instead of
                # the (slow, serial) gpsimd partition_broadcast -- keeps the
                # PE warm going into the final out-projection
                for j in range(2):
                    r_bf = small.tile([1, QCW], BF16, name="r_bf", bufs=3)
                    nc.scalar.activation(
                        r_bf, r_sbs[j], mybir.ActivationFunctionType.Copy)
                    rbc_ps = pjp.tile([P, QCW], F32, name="pj_ps")
                    nc.tensor.matmul(
                        rbc_ps[0:HD, :], lhsT=ones64, rhs=r_bf,
                        start=True, stop=True)
                    rbcs.append(rbc_ps[0:HD, :])
            else:
                for j in range(2):
                    rbc = rpool.tile([HD, QCW], F32, name="rbc")
                    nc.gpsimd.partition_broadcast(out_ap=rbc, in_ap=r_sbs[j])
                    rbcs.append(rbc)
            doler.pump(k=2)
            for j in range(2):
                nc.vector.tensor_mul(
                    CT[pr][HD * j:HD * (j + 1), QCW * qc:QCW * (qc + 1)],
                    ctxs[j][0:HD, :], rbcs[j])

        # ---- stage loop ----
        for n in range(NQC):
            qc = n
            # pr0 block: fillers = qproj m0, kproj m0, vproj 0-3, qproj m1
            f0 = Doler([
                gen_qkproj(n, wq_sb, wq_aug, xq_sb, xq_aug, QT, 0),
                gen_qkproj(n, wk_sb, wk_aug, xk_sb, xk_aug, KT, 0),
                gen_vproj(n, 0), gen_vproj(n, 1),
                gen_vproj(n, 2), gen_vproj(n, 3),
                gen_qkproj(n, wq_sb, wq_aug, xq_sb, xq_aug, QT, 1),
            ])
            # minimum cumulative quanta before tile t (absolute indices)
            needs0 = {0: 2, 4 * n: 6, 4 * n + 1: 8, 4 * n + 2: 10,
                      4 * n + 3: 12, "total": 14}
            hooks0 = None
            if n == 0:
                def early_xq_c1():
                    # dummy gpsimd op depending on QT[0] gates the dispatch
                    # of the xq chunk-1 load to ~when stage-0 q-proj is done
                    dummy = small.tile([P, HEADS], BF16, name="dummy")
                    nc.gpsimd.partition_broadcast(
                        out_ap=dummy, in_ap=QT[0][0:1, 0:HEADS])
                    load_x_chunk(1, keys="q")
                hooks0 = {0: early_xq_c1}
            gen_attn(qc, 0, f0, needs0, hooks=hooks0)
            f0.drain()
            if n == 0:
                load_x_chunk(1, keys="kv")
            elif n == 1:
                load_x_chunk(3)

            # pr1 block: fillers = kproj m1, outproj(qc-1)
            gens1 = [gen_qkproj(n, wk_sb, wk_aug, xk_sb, xk_aug, KT, 1)]
            if n >= 1:
                gens1 += [gen_outproj(n - 1, mq) for mq in range(4)]
            f1 = Doler(gens1)
            needs1 = {4 * n: 2, "total": 2 + (8 if n >= 1 else 0)}
            if n == NQC - 1:
                needs1["hold"] = 12
            gen_attn(qc, 1, f1, needs1)
            f1.drain()
            if n == 0:
                load_ow()
                load_x_chunk(2)

        for mq in range(4):
            for _ in gen_outproj(NQC - 1, mq, final=True):
                pass

    if compile_:
        nc.compile()
    return nc


def build_nc(mode: str, compile_: bool = True, probes: bool = False,
             has_bias: bool = False) -> bass.Bass:
    """mode in {causal, nomask, generic}; causal uses the pipelined build."""
    if mode == "causal" and not probes:
        return build_nc_causal(compile_=compile_, has_bias=has_bias)
    nc = bacc.Bacc("TRN2", target_bir_lowering=False, debug=False)
    prb = {}
    if probes:
        for nm, shape in (("p_qt", [P, S]), ("p_kt", [P, S]),
                          ("p_va", [P, HEADS * (HD + 1)]), ("p_ct", [P, S])):
            prb[nm] = nc.dram_tensor(nm, shape, F32, kind="ExternalOutput").ap()
    xq = nc.dram_tensor("xqT", [D + 1, S], BF16, kind="ExternalInput").ap()
    xk = nc.dram_tensor("xkT", [D + 1, S], BF16, kind="ExternalInput").ap()
    xv = nc.dram_tensor("xvT", [D + 1, S], BF16, kind="ExternalInput").ap()
    wq = nc.dram_tensor("wqT", [D + 1, M_DIM], BF16, kind="ExternalInput").ap()
    wk = nc.dram_tensor("wkT", [D + 1, M_DIM], BF16, kind="ExternalInput").ap()
    wv = nc.dram_tensor("wvT", [D + 1, M_DIM], BF16, kind="ExternalInput").ap()
    ow = nc.dram_tensor("owT", [M_DIM, D], BF16, kind="ExternalInput").ap()
    btri = nc.dram_tensor("btri", [P, P], F32, kind="ExternalInput").ap()
    bfull = None
    if mode == "generic":
        bfull = nc.dram_tensor("biasT", [S, S], F32, kind="ExternalInput").ap()
    out = nc.dram_tensor("out", [S, D], BF16, kind="ExternalOutput").ap()

    with tile.TileContext(nc) as tc, ExitStack() as ctx:
        consts = ctx.enter_context(tc.tile_pool(name="consts", bufs=1))
        xpool = ctx.enter_context(tc.tile_pool(name="xpool", bufs=26))
        qkv = ctx.enter_context(tc.tile_pool(name="qkv", bufs=1))
        ppool = ctx.enter_context(tc.tile_pool(name="ppool", bufs=6))
        bpool = ctx.enter_context(tc.tile_pool(name="bpool", bufs=2))
        small = ctx.enter_context(tc.tile_pool(name="small", bufs=4))
        outp = ctx.enter_context(tc.tile_pool(name="outp", bufs=2))
        spool = ctx.enter_context(tc.tile_pool(name="spsum", bufs=3, space="PSUM"))
        cpool = ctx.enter_context(tc.tile_pool(name="cpsum", bufs=2, space="PSUM"))

        # ---- resident weights ----
        def load_w(ap_dram, nm):
            tiles = []
            for kc in range(KC):
                t = consts.tile([P, M_DIM], BF16, name=f"{nm}{kc}")
                nc.scalar.dma_start(out=t, in_=ap_dram[P * kc:P * (kc + 1), :])
                tiles.append(t)
            aug = None
            if has_bias:
                aug = consts.tile([1, M_DIM], BF16, name=f"{nm}_aug")
                nc.sync.dma_start(out=aug, in_=ap_dram[D:D + 1, :])
            return tiles, aug

        wq_sb, wq_aug = load_w(wq, "wq")
        wk_sb, wk_aug = load_w(wk, "wk")
        wv_sb, wv_aug = load_w(wv, "wv")
        ow_sb = []
        for pr in range(2):
            t = consts.tile([P, D], BF16, name=f"ow{pr}")
            nc.scalar.dma_start(out=t, in_=ow[P * pr:P * (pr + 1), :])
            ow_sb.append(t)
        btri_sb = consts.tile([P, P], F32, name="btri_sb")
        nc.scalar.dma_start(out=btri_sb, in_=btri)
        ones4 = consts.tile([P, HEADS], F32, name="ones4")
        nc.vector.memset(ones4, 1.0)

        QT = [qkv.tile([P, S], BF16, name=f"QT{pr}") for pr in range(2)]
        KT = [qkv.tile([P, S], BF16, name=f"KT{pr}") for pr in range(2)]
        CT = [qkv.tile([P, S], BF16, name=f"CT{pr}") for pr in range(2)]
        VA = [qkv.tile([P, HEADS, HD + 1], BF16, name=f"VA{t}") for t in range(NKT)]

        pending_outproj = []

        def emit_outproj(qc):
            for mq in range(QCW // P):
                out_sb = outp.tile([P, D], BF16, name="out_sb")
                q0 = QCW * qc + P * mq
                for ne in range(2):
                    o_ps = spool.tile([P, 2, QCW], F32, name="s_ps")[:, 0, :]
                    for pr2 in range(2):
                        nc.tensor.matmul(
                            o_ps,
                            lhsT=CT[pr2][:, q0:q0 + P],
                            rhs=ow_sb[pr2][:, QCW * ne:QCW * (ne + 1)],
                            start=(pr2 == 0), stop=(pr2 == 1))
                    nc.vector.tensor_copy(out_sb[:, QCW * ne:QCW * (ne + 1)], o_ps)
                nc.gpsimd.dma_start(out=out[q0:q0 + P, :], in_=out_sb)

        def load_pieces(xap, n):
            """8 [128, 1024] pieces of x.T covering the q columns of stages
            n and n+1, plus the bias ones-row piece."""
            ps = []
            for kc in range(KC):
                xt = xpool.tile([P, 2 * QCW], BF16, name="xt")
                nc.sync.dma_start(
                    out=xt,
                    in_=xap[P * kc:P * (kc + 1), QCW * n:QCW * (n + 2)])
                ps.append(xt)
            aug = None
            if has_bias:
                aug = small.tile([1, 2 * QCW], BF16, name="xaug", bufs=3)
                nc.sync.dma_start(
                    out=aug, in_=xap[D:D + 1, QCW * n:QCW * (n + 2)])
            return ps, aug

        xh = {}
        for n in range(NQC):
            # ---- stage n projections: q/k columns + v rows [512n, 512n+512) ----
            if n % 2 == 0:
                xh["q"] = load_pieces(xq, n)
                xh["k"] = load_pieces(xk, n)
                xh["v"] = load_pieces(xv, n)
            hs = (n % 2) * QCW  # column offset within the 2-stage piece
            for key, w_sb, w_aug, dest in (("q", wq_sb, wq_aug, QT),
                                           ("k", wk_sb, wk_aug, KT)):
                x_p, x_a = xh[key]
                for m in range(2):
                    ps = spool.tile([P, 2, QCW], F32, name="s_ps")
                    for kc in range(KC):
                        nc.tensor.matmul(
                            ps[:, 0, :],
                            lhsT=w_sb[kc][:, P * m:P * (m + 1)],
                            rhs=x_p[kc][:, hs:hs + QCW],
                            start=(kc == 0),
                            stop=(not has_bias and kc == KC - 1))
                    if has_bias:
                        nc.tensor.matmul(
                            ps[:, 0, :],
                            lhsT=w_aug[0:1, P * m:P * (m + 1)],
                            rhs=x_a[0:1, hs:hs + QCW],
                            start=False, stop=True)
                    nc.vector.tensor_copy(
                        dest[m][:, QCW * n:QCW * (n + 1)], ps[:, 0, :])
            xv_p, xv_a = xh["v"]
            for mv in range(4):
                m = 4 * n + mv
                ps = spool.tile([P, 2, QCW], F32, name="s_ps")
                for kc in range(KC):
                    nc.tensor.matmul(
                        ps[:, 0, 0:M_DIM],
                        lhsT=xv_p[kc][:, hs + P * mv:hs + P * (mv + 1)],
                        rhs=wv_sb[kc],
                        start=(kc == 0),
                        stop=(not has_bias and kc == KC - 1))
                if has_bias:
                    nc.tensor.matmul(
                        ps[:, 0, 0:M_DIM],
                        lhsT=xv_a[0:1, hs + P * mv:hs + P * (mv + 1)],
                        rhs=wv_aug,
                        start=False, stop=True)
                nc.vector.tensor_copy(
                    VA[m][:, :, 0:HD],
                    ps[:, 0, 0:M_DIM].rearrange("p (h d) -> p h d", h=HEADS))
                nc.vector.tensor_copy(
                    VA[m][:, :, HD:HD + 1],
                    ones4.rearrange("p (h o) -> p h o", o=1))
            if pending_outproj:
                emit_outproj(pending_outproj.pop(0))
            if probes and n == NQC - 1:
                nc.sync.dma_start(out=prb["p_qt"].bitcast(BF16)[:, 0:S], in_=QT[0])
                nc.sync.dma_start(out=prb["p_kt"].bitcast(BF16)[:, 0:S], in_=KT[0])
                nc.sync.dma_start(
                    out=prb["p_va"].bitcast(BF16)[:, 0:HEADS * (HD + 1)],
                    in_=VA[0].rearrange("p h d -> p (h d)"))

            # ---- stage n attention (q chunk n) ----
            qc = n
            for pr in range(2):
                nt = 4 * qc + 4 if mode == "causal" else NKT
                ctxs = [cpool.tile([HD + 1, QCW], F32, name="ctx_ps")
                        for _ in range(2)]
                queues = ([], [])

                def flush_ctx(j):
                    t0, p0, o0 = queues[j].pop(0)
                    nc.tensor.matmul(
                        ctxs[j][:, o0:],
                        lhsT=VA[t0][:, 2 * pr + j, :],
                        rhs=p0[:, j, o0:],
                        start=(t0 == 0), stop=(t0 == nt - 1),
                        skip_group_check=True)

                for t in range(nt):
                    o = max(0, P * t - QCW * qc) if mode == "causal" else 0
                    s_ps = spool.tile([P, 2, QCW], F32, name="s_ps")
                    for j in range(2):
                        nc.tensor.matmul(
                            s_ps[:, j, o:],
                            lhsT=KT[pr][HD * j:HD * (j + 1), P * t:P * (t + 1)],
                            rhs=QT[pr][HD * j:HD * (j + 1),
                                       QCW * qc + o:QCW * (qc + 1)],
                            start=True, stop=True,
                            tile_position=(HD * j, 0))
                    if mode == "causal" and t >= 4 * qc:
                        nc.vector.tensor_add(
                            s_ps[:, :, o:o + P],
                            s_ps[:, :, o:o + P],
                            btri_sb.rearrange("p (a q) -> p a q", a=1)
                            .to_broadcast([P, 2, P]))
                    elif mode == "generic":
                        bt = bpool.tile([P, QCW], F32, name="bt")
                        nc.sync.dma_start(
                            out=bt,
                            in_=bfull[P * t:P * (t + 1), QCW * qc:QCW * (qc + 1)])
                        nc.vector.tensor_add(
                            s_ps, s_ps,
                            bt.rearrange("p (a q) -> p a q", a=1)
                            .to_broadcast([P, 2, QCW]))
                    p_sb = ppool.tile([P, 2, QCW], BF16, name="p_sb")
                    nc.scalar.activation(
                        p_sb[:, :, o:], s_ps[:, :, o:], EXPF, scale=0.125)
                    for j in range(2):
                        queues[j].append((t, p_sb, o))
                    for j in range(2):
                        if len(queues[j]) > 2:
                            flush_ctx(j)
                for j in range(2):
                    while queues[j]:
                        flush_ctx(j)
                for j in range(2):
                    ctx_ps = ctxs[j]
                    l_sb = small.tile([1, QCW], F32, name="l_sb", bufs=3)
                    nc.vector.tensor_copy(l_sb, ctx_ps[HD:HD + 1, :])
                    r_sb = small.tile([1, QCW], F32, name="r_sb", bufs=3)
                    nc.vector.reciprocal_approx_fast(out=r_sb, in_=l_sb)
                    rbc = ppool.tile([HD, QCW], F32, name="rbc", bufs=2)
                    nc.gpsimd.partition_broadcast(out_ap=rbc, in_ap=r_sb)
                    nc.vector.tensor_mul(
                        CT[pr][HD * j:HD * (j + 1), QCW * qc:QCW * (qc + 1)],
                        ctx_ps[0:HD, :], rbc)

            pending_outproj.append(qc)
        emit_outproj(pending_outproj.pop(0))
        if probes:
            nc.sync.dma_start(out=prb["p_ct"].bitcast(BF16)[:, 0:S], in_=CT[0])

    if compile_:
        nc.compile()
    return nc


def _get_nc(mode, has_bias):
    key = (mode, has_bias)
    if key not in _NC_CACHE:
        _NC_CACHE[key] = build_nc(mode, has_bias=has_bias)
    return _NC_CACHE[key]


def _tri_bias():
    g = np.arange(P, dtype=np.int64)
    return np.where(g[None, :] < g[:, None], np.float32(NEG), np.float32(0.0))


def host_prep(query, key, value, attn_mask, q_w, q_b, k_w, k_b, v_w, v_b, o_w, o_b):
    """Build (mode, in_maps) for the 8 cores."""
    mask = np.asarray(attn_mask).astype(bool)
    if np.array_equal(mask, np.triu(np.ones((S, S), bool), 1)):
        mode = "causal"
    elif not mask.any():
        mode = "nomask"
    else:
        mode = "generic"

    import ml_dtypes
    bf16 = ml_dtypes.bfloat16
    ones_row = np.ones((1, S), bf16)

    def prep_x(x):
        return np.vstack([np.ascontiguousarray(x.T).astype(bf16), ones_row])

    xs = {}
    for b in range(2):
        xs[b] = (prep_x(np.asarray(query)[b]), prep_x(np.asarray(key)[b]),
                 prep_x(np.asarray(value)[b]))

    tri = _tri_bias()
    biasT = None
    if mode == "generic":
        biasT = np.ascontiguousarray(
            np.where(mask, np.float32(NEG), np.float32(0.0)).T)

    def prep_w(w, bvec, sl):
        return np.vstack([
            np.ascontiguousarray(np.asarray(w)[sl].T).astype(bf16),
            np.asarray(bvec)[sl][None, :].astype(bf16)])

    in_maps = []
    for c in range(8):
        b, g = divmod(c, 4)
        sl = slice(M_DIM * g, M_DIM * (g + 1))
        m = {
            "xqT": xs[b][0], "xkT": xs[b][1], "xvT": xs[b][2],
            "wqT": prep_w(q_w, q_b, sl),
            "wkT": prep_w(k_w, k_b, sl),
            "wvT": prep_w(v_w, v_b, sl),
            "owT": np.ascontiguousarray(np.asarray(o_w)[:, sl].T).astype(bf16),
            "btri": tri,
        }
        if mode == "generic":
            m["biasT"] = biasT
        in_maps.append(m)
    return mode, in_maps


def kernel(**inputs) -> np.ndarray:
    global LAST_RESULTS
    from concourse.bass_utils import run_bass_kernel_spmd

    mode, in_maps = host_prep(**inputs)
    has_bias = any(
        np.asarray(inputs[k]).any() for k in ("q_b", "k_b", "v_b"))
    nc = _get_nc(mode, has_bias)
    res = run_bass_kernel_spmd(nc, in_maps, core_ids=list(range(8)), trace=TRACE)
    LAST_RESULTS = res
    parts = [np.asarray(res.results[c]["out"]).astype(np.float32)
             for c in range(8)]
    o_b = np.asarray(inputs["o_b"]).astype(np.float32)
    out = np.stack([
        parts[0] + parts[1] + parts[2] + parts[3],
        parts[4] + parts[5] + parts[6] + parts[7],
    ], axis=0) + o_b[None, None, :]
    return out.astype(np.float32)
